# revision 1
# baseline (speedup 1.0000x reference)
"""Cross-attention head (B=4, T=S=4096, C=1024, HS=64) on 8 TRN2 NeuronCores.

Sharding: core i handles batch b = i//2, query-half th = i%2 (2048 query rows).
Each core gets a transposed slice xT [C, 2048] and its batch's encT [C, S]
(host-side layout prep), plus packed weights Wqq=[Wq|Wq], Wkv=[Wk|Wv],
Wvk=[Wv|Wk] ([C,128] each).

All activations/weights are cast to bf16 on the host: halves HBM->SBUF DMA
traffic and runs every matmul at 1 cycle/row (fp32 needs 4).  PSUM
accumulation and the softmax normalization stay fp32; output is fp32.

Per-core pipeline:
  qT2 [128, 2048] = (Wqq)^T @ xT           rows 0:64 = q^T, rows 64:128 = copy
  stream over s in 512-chunks, alternating Wkv / Wvk so that k^T lands on
  partitions 0:64 (even chunks) or 64:128 (odd chunks); v^T on the other half.
  v^T chunks are transposed on the PE (identity matmul) into v_aug [128s, 65]
  tiles (col 64 = 1.0, giving the softmax denominator for free).
  scoresT [s,t] = kT^T_block @ qT2: two row-packed matmuls (tile rows 0:63 and
  64:127) run concurrently; one ACT Exp (scale=1/8) evacuates both PSUM banks
  to a bf16 U tile.
  PV: po[65, t] += v_aug^T @ U accumulated over all 32 s-blocks per t-chunk.
  Tail: po -> transpose -> divide rows by Z (col 64) -> out [2048, 64] fp32.
"""

import numpy as np

B, T, S, C, HS = 4, 4096, 4096, 1024, 64
NCORE = 8
TSH = T // 2            # 2048 query rows per core
KT = C // 128           # 8 contraction k-tiles
NTCH = TSH // 512       # 4 t-chunks
NCP = S // 1024         # 4 s-chunk pairs (each pair = 2x 512 keys)
SCALE = HS ** -0.5

_CACHE = {}

# build options for the shipped kernel (see _build); test.py reuses this
# so its timing measures the same configuration kernel() runs.
BEST_KW = {"wide": True}


def _build(reps=1, bunch=False, noexp=False, nokv=False, pvlast=False,
           wide=False, deep=False, pqps=False):
    # deep (wide mode only): exp->PV lag of 2 tiles instead of 1 and kv
    # thunks spread across the whole pair instead of the first 12 granules.
    # wide: exp tiles of [128, 3*512] (43 ACT instructions instead of 64,
    # cutting the per-instruction ACT tax) — enabled by spilling each
    # (s-pair, t-chunk) PV partial sum from a 2-bank transient PSUM
    # rotation into SBUF accumulators via DVE adds, freeing the 4 standing
    # po banks.  Mutually exclusive with the attribution variants.
    # noexp/nokv/pvlast are TIMING-ONLY attribution variants (wrong
    # results): noexp replaces the softmax exp with a constant-ones U tile
    # (removes ACT from the pipeline); nokv reuses s-chunk pair 0's k/v for
    # every pair (removes enc DMA + kv projection beyond the first pair);
    # pvlast emits PV matmuls only for the final s-chunk pair (removes 3/4
    # of the PV load from the PE).
    import concourse.bass as bass
    import concourse.mybir as mybir
    from concourse import bacc
    from concourse.tile import TileContext
    from concourse.masks import make_identity

    f32 = mybir.dt.float32
    bf16 = mybir.dt.bfloat16
    Exp = mybir.ActivationFunctionType.Exp

    nc = bacc.Bacc("TRN2", target_bir_lowering=False, debug=False,
                   num_devices=NCORE)
    # Host-packed layouts: every DMA sees per-partition contiguous runs.
    # xT packed as [tch, p, k, 512], encT as [sch, p, k, 512],
    # weights as [p, k, 128], out as [tch, p, j, 64].
    xT = nc.dram_tensor("xT", [NTCH, 128, KT, 512], bf16, kind="ExternalInput")
    encT = nc.dram_tensor("encT", [S // 512, 128, KT, 512], bf16,
                          kind="ExternalInput")
    wqq = nc.dram_tensor("Wqq", [128, KT, 128], bf16, kind="ExternalInput")
    wkv = nc.dram_tensor("Wkv", [128, KT, 128], bf16, kind="ExternalInput")
    wvk = nc.dram_tensor("Wvk", [128, KT, 128], bf16, kind="ExternalInput")
    out = nc.dram_tensor("out", [NTCH, 128, 4, HS], f32,
                         kind="ExternalOutput")

    xT_v = xT[:]       # [4, 128, 8, 512]
    encT_v = encT[:]   # [8, 128, 8, 512]
    out_v = out[:]     # [4, 128, 4, 64]

    with TileContext(nc) as tc:
        from contextlib import ExitStack
        with ExitStack() as ctx:
            ep = ctx.enter_context
            wpool = ep(tc.tile_pool(name="w", bufs=1))
            qpool = ep(tc.tile_pool(name="qt", bufs=2))
            xtp = ep(tc.tile_pool(name="xt", bufs=3))
            encp = ep(tc.tile_pool(name="enc", bufs=6))
            ktp = ep(tc.tile_pool(name="kt", bufs=4))
            vtp = ep(tc.tile_pool(name="vt", bufs=2))
            vap = ep(tc.tile_pool(name="va", bufs=4))
            up = ep(tc.tile_pool(name="u", bufs=5 if deep else 4))
            otp = ep(tc.tile_pool(name="ot", bufs=2))
            obp = ep(tc.tile_pool(name="ob", bufs=2))
            rp = ep(tc.tile_pool(name="r", bufs=2))
            # PSUM narrow: po 4 banks + shared pool 2x[128,1024] = 4 -> 8
            # PSUM wide: ptv 2x[128,512] = 2 + shared 2x[128,1536] = 6 -> 8
            if wide:
                accp = ep(tc.tile_pool(name="acc", bufs=2))
                ptvp = ep(tc.tile_pool(name="ptv", bufs=2, space="PSUM"))
            else:
                pop = ep(tc.tile_pool(name="po", bufs=1, space="PSUM"))
            psp = ep(tc.tile_pool(name="ps", bufs=2, space="PSUM"))

            # static tiles
            ident = wpool.tile([128, 128], bf16, tag="ident")
            make_identity(nc, ident[:])
            u_ones = None
            if noexp:
                u_ones = wpool.tile([128, 1024], bf16, tag="u1")
                nc.gpsimd.memset(u_ones[:], 1.0)
            identf = wpool.tile([128, 128], f32, tag="identf")
            make_identity(nc, identf[:])
            w_sb = {}
            for name, dram in (("qq", wqq), ("kv", wkv), ("vk", wvk)):
                wt = wpool.tile([128, KT * 128], bf16, tag=f"w{name}")
                nc.sync.dma_start(
                    out=wt[:].rearrange("p (k m) -> p k m", k=KT),
                    in_=dram[:])
                w_sb[name] = wt[:].rearrange("p (k m) -> p k m", k=KT)

            def phase_q_thunks(qt2):
                """Per-t-chunk thunks computing qT2 = [Wq|Wq]^T @ xT.
                Interleaved into the previous rep's final s-pair (which has
                no kv thunks), hiding the Q projection under the exp
                stream."""
                def load_x(tch):
                    def f():
                        xt = xtp.tile([128, KT * 512], bf16, tag="xt")
                        xt3 = xt[:].rearrange("p (k n) -> p k n", k=KT)
                        nc.sync.dma_start(out=xt3, in_=xT_v[tch])
                        # wide mode: take pq from the ptv pool (its PV
                        # rotation has ~4µs slack/group) so the final pair
                        # carries the same 2 "ps"-pool steals as the others
                        if wide and not pqps:
                            pq = ptvp.tile([128, 512], f32, tag="ptv",
                                           name="pq")
                        else:
                            pq = psp.tile([128, 512], f32, tag="ps",
                                          name="pq")
                        for k in range(KT):
                            nc.tensor.matmul(pq[:], w_sb["qq"][:, k, :],
                                             xt3[:, k, :],
                                             start=(k == 0),
                                             stop=(k == KT - 1))
                        nc.vector.tensor_copy(
                            qt2[:, tch * 512:(tch + 1) * 512], pq[:])
                    return f
                return [load_x(tch) for tch in range(NTCH)]

            qt2_next = None
            cur = None          # s-pair-0 kv state, pipelined across reps
            for _rep in range(reps):
                if qt2_next is None:
                    qt2 = qpool.tile([128, TSH], bf16, tag="qt2")
                    for th in phase_q_thunks(qt2):
                        th()
                else:
                    qt2 = qt2_next
                qt2_next = qpool.tile([128, TSH], bf16, tag="qt2")

                # ---- Phase S: stream s-chunk pairs
                po = None
                if not wide:
                    po = [pop.tile([128, 512], f32, tag=f"po{t}",
                                   name=f"po{t}")
                          for t in range(NTCH)]

                def make_kv_thunks(cp):
                    """Emit-later closures for loading/projecting s-chunk pair
                    cp.  Returns (thunks, kt_tiles, va_views).

                    All of one parity's PSUM traffic (kv projection + the 4
                    v-transposes) is packed into a single [128,1024] tile
                    (proj in bank cols 0:512, transposes at 512+65j), so a
                    pair costs 2 "ps" rotations instead of 10 and barely
                    disturbs the scores/exp double-buffer."""
                    kts, vas = [None, None], [None, None]
                    pw = [None, None]
                    thunks = []

                    def load(par):
                        def f():
                            sch = 2 * cp + par
                            enc = encp.tile([128, KT * 512], bf16, tag="enc")
                            enc3 = enc[:].rearrange("p (k n) -> p k n", k=KT)
                            nc.sync.dma_start(out=enc3, in_=encT_v[sch])
                            t1 = psp.tile([128, 1024], f32, tag="ps",
                                          name="pkv")
                            pkv = t1[:, 0:512]
                            wname = "kv" if par == 0 else "vk"
                            for k in range(KT):
                                nc.tensor.matmul(pkv, w_sb[wname][:, k, :],
                                                 enc3[:, k, :],
                                                 start=(k == 0),
                                                 stop=(k == KT - 1))
                            kt = ktp.tile([128, 512], bf16, tag="kt")
                            vt = vtp.tile([128, 512], bf16, tag="vt")
                            if par == 0:   # kT on rows 0:64, vT on rows 64:128
                                nc.vector.tensor_copy(kt[0:64, :], pkv[0:64, :])
                                nc.vector.tensor_copy(vt[64:128, :],
                                                      pkv[64:128, :])
                            else:
                                nc.vector.tensor_copy(kt[64:128, :],
                                                      pkv[64:128, :])
                                nc.vector.tensor_copy(vt[0:64, :], pkv[0:64, :])
                            va = vap.tile([128, 4 * 65], bf16, tag="va")
                            va3 = va[:].rearrange("p (j m) -> p j m", j=4)
                            nc.gpsimd.memset(va3[:, :, 64:65], 1.0)
                            kts[par] = kt
                            vas[par] = (va3, vt)
                            pw[par] = t1
                            # inline v-transposes: keeps t1's "ps"-rotation
                            # steal compact (one short window per parity)
                            rows = (slice(64, 128) if par == 0
                                    else slice(0, 64))
                            for j in range(4):
                                pvt = t1[:, 512 + 65 * j:512 + 65 * j + 65]
                                nc.tensor.matmul(
                                    pvt[:, 0:64],
                                    vt[rows, j * 128:(j + 1) * 128],
                                    ident[rows, rows],
                                    start=True, stop=True,
                                    skip_group_check=True)
                                nc.vector.tensor_copy(va3[:, j, 0:64],
                                                      pvt[:, 0:64])
                        return f

                    for par in range(2):
                        thunks.append(load(par))
                    return thunks, kts, vas

                def emit_pv(prev):
                    """PV matmuls for a previously-exp'd pair (one-pair SW
                    pipeline keeps the PE from stalling on the current exp)."""
                    u, pvas, pcp, ptch, ppb = prev
                    if pvlast and pcp != NCP - 1:
                        return
                    first = ((NCP - 1 if pvlast else 0) == pcp and ppb == 0)
                    last = (pcp == NCP - 1 and ppb == 3)
                    nc.tensor.matmul(po[ptch][0:65, :],
                                     pvas[0][:, ppb, :], u[:, 0:512],
                                     start=first, stop=False,
                                     skip_group_check=True)
                    nc.tensor.matmul(po[ptch][0:65, :],
                                     pvas[1][:, ppb, :], u[:, 512:1024],
                                     start=False, stop=last,
                                     skip_group_check=True)

                if cur is None:
                    cur = make_kv_thunks(0)
                    for th in cur[0]:
                        th()

                if wide:
                    # granule = one [128s x 512t] scores block, keyed
                    # (cp, tch, pb, par); 3 granules share one exp tile.
                    acc = [accp.tile([128, 512], f32, tag=f"acc{t}",
                                     name=f"acc{t}") for t in range(NTCH)]
                    open_ptv = {}

                    def tail_tch(tch):
                        """Normalize + store one t-chunk; emitted as soon as
                        its final PV spill lands so it overlaps the rest of
                        the exp stream instead of trailing the rep."""
                        ob = obp.tile([128, 4 * 64], f32, tag="ob")
                        ob3 = ob[:].rearrange("p (j d) -> p j d", j=4)
                        for j in range(4):
                            pt = psp.tile([128, 65], f32, tag="ps",
                                          name="pt")
                            nc.tensor.matmul(
                                pt[:],
                                acc[tch][0:65, j * 128:(j + 1) * 128],
                                identf[0:65, 0:65],
                                start=True, stop=True)
                            rr = rp.tile([128, 1], f32, tag="r")
                            nc.vector.reciprocal(rr[:], pt[:, 64:65])
                            nc.vector.tensor_scalar_mul(ob3[:, j, :],
                                                        pt[:, 0:64], rr[:])
                        nc.sync.dma_start(out=out_v[tch], in_=ob3)

                    def emit_pv_tile(entry):
                        u, gr_list = entry
                        for j, (gcp, gtch, gpb, gpar, gva) in \
                                enumerate(gr_list):
                            first = (gpb == 0 and gpar == 0)
                            if first:
                                open_ptv[(gcp, gtch)] = ptvp.tile(
                                    [128, 512], f32, tag="ptv",
                                    name="ptv")
                            ptv = open_ptv[(gcp, gtch)]
                            stop = (gpb == 3 and gpar == 1)
                            nc.tensor.matmul(
                                ptv[0:65, :], gva[:, gpb, :],
                                u[:, 512 * j:512 * (j + 1)],
                                start=first, stop=stop,
                                skip_group_check=True)
                            if stop:
                                del open_ptv[(gcp, gtch)]
                                if gcp == 0:
                                    nc.vector.tensor_copy(
                                        acc[gtch][0:65, :], ptv[0:65, :])
                                else:
                                    nc.vector.tensor_add(
                                        acc[gtch][0:65, :],
                                        acc[gtch][0:65, :], ptv[0:65, :])

                    GW = 3
                    n_gr = NCP * NTCH * 8
                    pend = []
                    tile_ps = None
                    gr_list = []
                    kts = vas = None
                    nxt = None
                    n_thunks = ti = 0
                    for gi in range(n_gr):
                        cp, w = divmod(gi, 32)
                        tch, r = divmod(w, 8)
                        pb, par = divmod(r, 2)
                        if w == 0:
                            _, kts, vas = cur
                            if cp + 1 < NCP:
                                nxt = make_kv_thunks(cp + 1)
                            else:
                                nxt0 = make_kv_thunks(0)
                                nxt = (phase_q_thunks(qt2_next) + nxt0[0],
                                       nxt0[1], nxt0[2])
                            n_thunks = len(nxt[0])
                            ti = 0
                        if tile_ps is None:
                            tile_ps = psp.tile([128, GW * 512], f32,
                                               tag="ps")
                            gr_list = []
                        col = 512 * len(gr_list)
                        rows = slice(0, 64) if par == 0 else slice(64, 128)
                        nc.tensor.matmul(
                            tile_ps[:, col:col + 512],
                            kts[par][rows, pb * 128:(pb + 1) * 128],
                            qt2[rows, tch * 512:(tch + 1) * 512],
                            start=True, stop=True)
                        gr_list.append((cp, tch, pb, par, vas[par][0]))
                        if len(gr_list) == GW or gi == n_gr - 1:
                            wcols = 512 * len(gr_list)
                            u = up.tile([128, GW * 512], bf16, tag="u")
                            nc.scalar.activation(u[:, 0:wcols],
                                                 tile_ps[:, 0:wcols],
                                                 Exp, scale=SCALE)
                            pend.append((u, gr_list))
                            tile_ps = None
                            if len(pend) > (2 if deep else 1):
                                emit_pv_tile(pend.pop(0))
                        tgt = min(n_thunks,
                                  (n_thunks * (w + 1)) // (24 if deep else 12))
                        while ti < tgt:
                            nxt[0][ti]()
                            ti += 1
                        if w == 31:
                            while ti < n_thunks:
                                nxt[0][ti]()
                                ti += 1
                            cur = nxt
                    for entry in pend:
                        emit_pv_tile(entry)
                    # tail: normalize straight from the SBUF accumulators.
                    # (Emitting each tail inline right after its final PV
                    # spill was tried and measured ~5µs SLOWER: the 16 extra
                    # "ps"-pool steals land in the rep-end region that
                    # already interleaves next-rep Q and kv thunks.)
                    for tch in range(NTCH):
                        tail_tch(tch)
                    continue

                prevs = []      # depth-2 exp->PV pipeline: PV never waits exp
                for cp in range(NCP):
                    _, kts, vas = cur
                    if nokv:
                        nxt = ([], kts, vas)
                    elif cp + 1 < NCP:
                        nxt = make_kv_thunks(cp + 1)
                    else:
                        # final pair: interleave next rep's Q projection and
                        # its s-pair 0 load instead of kv thunks
                        nxt0 = make_kv_thunks(0)
                        nxt = (phase_q_thunks(qt2_next) + nxt0[0],
                               nxt0[1], nxt0[2])
                    n_thunks = len(nxt[0])
                    ti = 0
                    pair_idx = 0
                    for tch in range(NTCH):
                        for pb in range(4):
                            ps = psp.tile([128, 1024], f32, tag="ps")
                            nc.tensor.matmul(
                                ps[:, 0:512],
                                kts[0][0:64, pb * 128:(pb + 1) * 128],
                                qt2[0:64, tch * 512:(tch + 1) * 512],
                                start=True, stop=True)
                            nc.tensor.matmul(
                                ps[:, 512:1024],
                                kts[1][64:128, pb * 128:(pb + 1) * 128],
                                qt2[64:128, tch * 512:(tch + 1) * 512],
                                start=True, stop=True)
                            if noexp:
                                u = u_ones
                            else:
                                u = up.tile([128, 1024], bf16, tag="u")
                                nc.scalar.activation(u[:], ps[:], Exp,
                                                     scale=SCALE)
                            prevs.append((u, (vas[0][0], vas[1][0]),
                                          cp, tch, pb))
                            if len(prevs) > 2:
                                emit_pv(prevs.pop(0))
                            pair_idx += 1
                            # interleave next chunk-pair's kv work over the
                            # first half of this pair (bunch: defer it all to
                            # the pair boundary)
                            target = (0 if bunch
                                      else min(n_thunks,
                                               (n_thunks * pair_idx) // 8))
                            while ti < target:
                                nxt[0][ti]()
                                ti += 1
                    while ti < n_thunks:
                        nxt[0][ti]()
                        ti += 1
                    cur = nxt
                for p in prevs:
                    emit_pv(p)

                # ---- Tail: normalize + store
                for tch in range(NTCH):
                    ot = otp.tile([128, 512], f32, tag="ot")
                    nc.vector.tensor_copy(ot[0:65, :], po[tch][0:65, :])
                    ob = obp.tile([128, 4 * 64], f32, tag="ob")
                    ob3 = ob[:].rearrange("p (j d) -> p j d", j=4)
                    for j in range(4):
                        pt = psp.tile([128, 65], f32, tag="ps", name="pt")
                        nc.tensor.matmul(pt[:],
                                         ot[0:65, j * 128:(j + 1) * 128],
                                         identf[0:65, 0:65],
                                         start=True, stop=True)
                        r = rp.tile([128, 1], f32, tag="r")
                        nc.vector.reciprocal(r[:], pt[:, 64:65])
                        nc.vector.tensor_scalar_mul(ob3[:, j, :],
                                                    pt[:, 0:64], r[:])
                    nc.sync.dma_start(out=out_v[tch], in_=ob3)

    nc.compile()
    return nc


def _get_nc(reps=1, **kw):
    key = (reps, tuple(sorted(kw.items())))
    if key not in _CACHE:
        _CACHE[key] = _build(reps, **kw)
    return _CACHE[key]


def _bf16(a):
    import ml_dtypes
    return np.asarray(a).astype(ml_dtypes.bfloat16)


def _pack_act(a, nch):
    """[L, C] row-major -> [L/512, 128, KT, 512] (chunk, partition, k, col)."""
    return np.ascontiguousarray(
        a.reshape(nch, 512, KT, 128).transpose(0, 3, 2, 1))


def _pack_w(w2):
    """[C, 128] -> [128, KT, 128]."""
    return np.ascontiguousarray(w2.reshape(KT, 128, 128).transpose(1, 0, 2))


def _prep_inputs(x, encode_out, Wq, Wk, Wv):
    x = _bf16(x)
    encode_out = _bf16(encode_out)
    Wq = _bf16(Wq)
    Wk = _bf16(Wk)
    Wv = _bf16(Wv)
    wqq = _pack_w(np.concatenate([Wq, Wq], axis=1))
    wkv = _pack_w(np.concatenate([Wk, Wv], axis=1))
    wvk = _pack_w(np.concatenate([Wv, Wk], axis=1))
    encTs = [_pack_act(encode_out[b], S // 512) for b in range(B)]
    in_maps = []
    for core in range(NCORE):
        b, th = divmod(core, 2)
        xTi = _pack_act(x[b, th * TSH:(th + 1) * TSH, :], NTCH)
        in_maps.append({"xT": xTi, "encT": encTs[b],
                        "Wqq": wqq, "Wkv": wkv, "Wvk": wvk})
    return in_maps


def kernel(x, encode_out, Wq, Wk, Wv):
    from concourse.bass_utils import run_bass_kernel_spmd
    nc = _get_nc(1, **BEST_KW)
    in_maps = _prep_inputs(x, encode_out, Wq, Wk, Wv)
    res = run_bass_kernel_spmd(nc, in_maps, list(range(NCORE)))
    out = np.empty((B, T, HS), dtype=np.float32)
    for core in range(NCORE):
        b, th = divmod(core, 2)
        o = res.results[core]["out"]            # [4, 128, 4, 64]
        out[b, th * TSH:(th + 1) * TSH] = (
            o.transpose(0, 2, 1, 3).reshape(TSH, HS))
    return out



# revision 6
# speedup vs baseline: 2.2852x; 2.2852x over previous
"""Cross-attention head (B=4, T=S=4096, C=1024, HS=64) on 8 TRN2 NeuronCores.

Sharding: core i handles batch b = i//2, query-half th = i%2 (2048 query rows).
Each core gets a transposed slice xT [C, 2048] and its batch's encT [C, S]
(host-side layout prep), plus packed weights Wqq=[Wq|Wq], Wkv=[Wk|Wv],
Wvk=[Wv|Wk] ([C,128] each).

All activations/weights are cast to bf16 on the host: halves HBM->SBUF DMA
traffic and runs every matmul at 1 cycle/row (fp32 needs 4).  PSUM
accumulation and the softmax normalization stay fp32; output is fp32.

Per-core pipeline:
  qT2 [128, 2048] = (Wqq)^T @ xT           rows 0:64 = q^T, rows 64:128 = copy
  stream over s in 512-chunks, alternating Wkv / Wvk so that k^T lands on
  partitions 0:64 (even chunks) or 64:128 (odd chunks); v^T on the other half.
  v^T chunks are transposed on the PE (identity matmul) into v_aug [128s, 65]
  tiles (col 64 = 1.0, giving the softmax denominator for free).
  scoresT [s,t] = kT^T_block @ qT2: two row-packed matmuls (tile rows 0:63 and
  64:127) run concurrently; one ACT Exp (scale=1/8) evacuates both PSUM banks
  to a bf16 U tile.
  PV: po[65, t] += v_aug^T @ U accumulated over all 32 s-blocks per t-chunk.
  Tail: po -> transpose -> divide rows by Z (col 64) -> out [2048, 64] fp32.
"""

import numpy as np

B, T, S, C, HS = 4, 4096, 4096, 1024, 64
NCORE = 8
TSH = T // 2            # 2048 query rows per core
KT = C // 128           # 8 contraction k-tiles
NTCH = TSH // 512       # 4 t-chunks
NCP = S // 1024         # 4 s-chunk pairs (each pair = 2x 512 keys)
SCALE = HS ** -0.5

_CACHE = {}

# build options for the shipped kernel (see _build); test.py reuses this
# so its timing measures the same configuration kernel() runs.
BEST_KW = {"wide": True}


def _build(reps=1, bunch=False, noexp=False, nokv=False, pvlast=False,
           wide=False, deep=False, pqps=False, esplit=None, pvdr=False):
    # esplit (new narrow path): fraction of exp tiles computed on ACT; the
    # rest use a DVE fast-exp bit trick (round(x*k+b) as uint viewed as
    # bf16/fp8 -- the Schraudolph approximation, ~1% rms weight error).
    # pvdr: U and va in fp8e4 and the two PV matmuls of a pair fused into
    # one DoubleRow matmul (2x PE throughput on the PV stream).
    # deep (wide mode only): exp->PV lag of 2 tiles instead of 1 and kv
    # thunks spread across the whole pair instead of the first 12 granules.
    # wide: exp tiles of [128, 3*512] (43 ACT instructions instead of 64,
    # cutting the per-instruction ACT tax) — enabled by spilling each
    # (s-pair, t-chunk) PV partial sum from a 2-bank transient PSUM
    # rotation into SBUF accumulators via DVE adds, freeing the 4 standing
    # po banks.  Mutually exclusive with the attribution variants.
    # noexp/nokv/pvlast are TIMING-ONLY attribution variants (wrong
    # results): noexp replaces the softmax exp with a constant-ones U tile
    # (removes ACT from the pipeline); nokv reuses s-chunk pair 0's k/v for
    # every pair (removes enc DMA + kv projection beyond the first pair);
    # pvlast emits PV matmuls only for the final s-chunk pair (removes 3/4
    # of the PV load from the PE).
    import concourse.bass as bass
    import concourse.mybir as mybir
    from concourse import bacc
    from concourse.tile import TileContext
    from concourse.masks import make_identity

    import math

    f32 = mybir.dt.float32
    bf16 = mybir.dt.bfloat16
    Exp = mybir.ActivationFunctionType.Exp

    newpath = esplit is not None
    if pvdr:
        assert newpath, "pvdr requires esplit"
    if newpath:
        assert not wide
        # qt2 is pre-scaled by C_PRE so the scores PSUM value is already in
        # "fast-exp units": u = psum + 127*128 (bf16) / 7*8 (fp8e4) rounds to
        # the bit pattern of ~exp(logit).  ACT granules undo the scaling via
        # the activation's free affine (scale=ln2/EBITS).
        EBITS = 8.0 if pvdr else 128.0
        C_PRE = SCALE * EBITS / math.log(2.0)
        ACT_SCALE = math.log(2.0) / EBITS
        # bias shifted down to center the Schraudolph chord error (mean-zero
        # relative error; otherwise ACT-exact and DVE-fastexp tiles in the
        # same softmax row disagree systematically by ~+4%)
        DVE_BIAS = (7.0 * 8.0 - 0.45) if pvdr else (127.0 * 128.0 - 7.2)
        DVE_CLAMP = 127.0 if pvdr else 32767.0
        u_dt = mybir.dt.float8e4 if pvdr else bf16
        u_bits = mybir.dt.uint8 if pvdr else mybir.dt.uint16

    nc = bacc.Bacc("TRN2", target_bir_lowering=False, debug=False,
                   num_devices=NCORE)
    # Host-packed layouts: every DMA sees per-partition contiguous runs.
    # xT packed as [tch, p, k, 512], encT as [sch, p, k, 512],
    # weights as [p, k, 128], out as [tch, p, j, 64].
    xT = nc.dram_tensor("xT", [NTCH, 128, KT, 512], bf16, kind="ExternalInput")
    encT = nc.dram_tensor("encT", [S // 512, 128, KT, 512], bf16,
                          kind="ExternalInput")
    wqq = nc.dram_tensor("Wqq", [128, KT, 128], bf16, kind="ExternalInput")
    wkv = nc.dram_tensor("Wkv", [128, KT, 128], bf16, kind="ExternalInput")
    wvk = nc.dram_tensor("Wvk", [128, KT, 128], bf16, kind="ExternalInput")
    out = nc.dram_tensor("out", [NTCH, 128, 4, HS], f32,
                         kind="ExternalOutput")

    xT_v = xT[:]       # [4, 128, 8, 512]
    encT_v = encT[:]   # [8, 128, 8, 512]
    out_v = out[:]     # [4, 128, 4, 64]

    with TileContext(nc) as tc:
        from contextlib import ExitStack
        with ExitStack() as ctx:
            ep = ctx.enter_context
            wpool = ep(tc.tile_pool(name="w", bufs=1))
            qpool = ep(tc.tile_pool(name="qt", bufs=2))
            xtp = ep(tc.tile_pool(name="xt", bufs=3))
            encp = ep(tc.tile_pool(name="enc", bufs=6))
            ktp = ep(tc.tile_pool(name="kt", bufs=4))
            vtp = ep(tc.tile_pool(name="vt", bufs=2))
            vap = ep(tc.tile_pool(name="va", bufs=4))
            up = ep(tc.tile_pool(name="u", bufs=5 if deep else 4))
            otp = ep(tc.tile_pool(name="ot", bufs=2))
            obp = ep(tc.tile_pool(name="ob", bufs=2))
            rp = ep(tc.tile_pool(name="r", bufs=2))
            # PSUM narrow: po 4 banks + shared pool 2x[128,1024] = 4 -> 8
            # PSUM wide: ptv 2x[128,512] = 2 + shared 2x[128,1536] = 6 -> 8
            if wide:
                accp = ep(tc.tile_pool(name="acc", bufs=2))
                ptvp = ep(tc.tile_pool(name="ptv", bufs=2, space="PSUM"))
            else:
                pop = ep(tc.tile_pool(name="po", bufs=1, space="PSUM"))
            psp = ep(tc.tile_pool(name="ps", bufs=2, space="PSUM"))

            # static tiles
            ident = wpool.tile([128, 128], bf16, tag="ident")
            make_identity(nc, ident[:])
            u_ones = None
            if noexp:
                u_ones = wpool.tile([128, 1024], bf16, tag="u1")
                nc.gpsimd.memset(u_ones[:], 1.0)
            identf = wpool.tile([128, 128], f32, tag="identf")
            make_identity(nc, identf[:])
            w_sb = {}
            for name, dram in (("qq", wqq), ("kv", wkv), ("vk", wvk)):
                wt = wpool.tile([128, KT * 128], bf16, tag=f"w{name}")
                nc.sync.dma_start(
                    out=wt[:].rearrange("p (k m) -> p k m", k=KT),
                    in_=dram[:])
                w_sb[name] = wt[:].rearrange("p (k m) -> p k m", k=KT)

            def phase_q_thunks(qt2):
                """Per-t-chunk thunks computing qT2 = [Wq|Wq]^T @ xT.
                Interleaved into the previous rep's final s-pair (which has
                no kv thunks), hiding the Q projection under the exp
                stream."""
                def load_x(tch):
                    def f():
                        xt = xtp.tile([128, KT * 512], bf16, tag="xt")
                        xt3 = xt[:].rearrange("p (k n) -> p k n", k=KT)
                        nc.sync.dma_start(out=xt3, in_=xT_v[tch])
                        # wide mode: take pq from the ptv pool (its PV
                        # rotation has ~4µs slack/group) so the final pair
                        # carries the same 2 "ps"-pool steals as the others
                        if wide and not pqps:
                            pq = ptvp.tile([128, 512], f32, tag="ptv",
                                           name="pq")
                        else:
                            pq = psp.tile([128, 512], f32, tag="ps",
                                          name="pq")
                        for k in range(KT):
                            nc.tensor.matmul(pq[:], w_sb["qq"][:, k, :],
                                             xt3[:, k, :],
                                             start=(k == 0),
                                             stop=(k == KT - 1))
                        if newpath:
                            nc.vector.tensor_scalar_mul(
                                qt2[:, tch * 512:(tch + 1) * 512], pq[:],
                                C_PRE)
                        else:
                            nc.vector.tensor_copy(
                                qt2[:, tch * 512:(tch + 1) * 512], pq[:])
                    return f
                return [load_x(tch) for tch in range(NTCH)]

            qt2_next = None
            cur = None          # s-pair-0 kv state, pipelined across reps
            for _rep in range(reps):
                if qt2_next is None:
                    qt2 = qpool.tile([128, TSH], bf16, tag="qt2")
                    for th in phase_q_thunks(qt2):
                        th()
                else:
                    qt2 = qt2_next
                qt2_next = qpool.tile([128, TSH], bf16, tag="qt2")

                # ---- Phase S: stream s-chunk pairs
                po = None
                if not wide:
                    po = [pop.tile([128, 512], f32, tag=f"po{t}",
                                   name=f"po{t}")
                          for t in range(NTCH)]

                def make_kv_thunks2(cp):
                    """New-path kv thunks: one merged [128,512] kvt copy per
                    s-chunk (k and v halves stay in their PSUM rows) and one
                    merged strided va copy per parity.  pvdr: both parities'
                    v^T blocks land in a single fp8 va tile laid out
                    [128, (pb,par), 80] so a DR matmul reads ko=par pairs."""
                    kts = [None, None]
                    vas = [None, None]
                    thunks = []

                    def load(par):
                        def f():
                            sch = 2 * cp + par
                            enc = encp.tile([128, KT * 512], bf16, tag="enc")
                            enc3 = enc[:].rearrange("p (k n) -> p k n", k=KT)
                            nc.sync.dma_start(out=enc3, in_=encT_v[sch])
                            t1 = psp.tile([128, 1024], f32, tag="ps",
                                          name="pkv")
                            pkv = t1[:, 0:512]
                            wname = "kv" if par == 0 else "vk"
                            for k in range(KT):
                                nc.tensor.matmul(pkv, w_sb[wname][:, k, :],
                                                 enc3[:, k, :],
                                                 start=(k == 0),
                                                 stop=(k == KT - 1))
                            kvt = ktp.tile([128, 512], bf16, tag="kt")
                            nc.vector.tensor_copy(kvt[:], pkv)
                            kts[par] = kvt
                            rows = (slice(64, 128) if par == 0
                                    else slice(0, 64))
                            for j in range(4):
                                pvt = t1[:, 512 + 65 * j:512 + 65 * j + 65]
                                nc.tensor.matmul(
                                    pvt[:, 0:64],
                                    kvt[rows, j * 128:(j + 1) * 128],
                                    ident[rows, rows],
                                    start=True, stop=True,
                                    skip_group_check=True)
                            src = t1[:, 512:512 + 4 * 65].rearrange(
                                "p (j m) -> p j m", j=4)[:, :, 0:64]
                            if pvdr:
                                if par == 0:
                                    va = vap.tile([128, 8 * 80], u_dt,
                                                  tag="va")
                                    va3 = va[:].rearrange(
                                        "p (j m) -> p j m", j=8)
                                    nc.gpsimd.memset(va3[:, :, 64:65], 1.0)
                                    vas[0] = vas[1] = va3
                                va3 = vas[0]
                                va4 = va3.rearrange(
                                    "p (j q) m -> p j q m", q=2)
                                nc.vector.tensor_copy(
                                    va4[:, :, par, 0:64], src)
                            else:
                                va = vap.tile([128, 4 * 65], bf16, tag="va")
                                va3 = va[:].rearrange("p (j m) -> p j m", j=4)
                                nc.gpsimd.memset(va3[:, :, 64:65], 1.0)
                                nc.vector.tensor_copy(va3[:, :, 0:64], src)
                                vas[par] = va3
                        return f

                    for par in range(2):
                        thunks.append(load(par))
                    return thunks, kts, vas

                def emit_pv2(prev):
                    u, pvas, pcp, ptch, ppb = prev
                    first = (pcp == 0 and ppb == 0)
                    last = (pcp == NCP - 1 and ppb == 3)
                    if pvdr:
                        u3 = u[:].rearrange("p (k n) -> p k n", k=2)
                        nc.tensor.matmul(
                            po[ptch][0:65, :],
                            pvas[0][:, 2 * ppb:2 * ppb + 2, 0:65], u3,
                            start=first, stop=last,
                            perf_mode=mybir.MatmulPerfMode.DoubleRow,
                            skip_group_check=True)
                    else:
                        nc.tensor.matmul(po[ptch][0:65, :],
                                         pvas[0][:, ppb, :], u[:, 0:512],
                                         start=first, stop=False,
                                         skip_group_check=True)
                        nc.tensor.matmul(po[ptch][0:65, :],
                                         pvas[1][:, ppb, :], u[:, 512:1024],
                                         start=False, stop=last,
                                         skip_group_check=True)

                if newpath:
                    if cur is None:
                        cur = make_kv_thunks2(0)
                        for th in cur[0]:
                            th()
                    prevs = []
                    ei_acc = 0.0
                    for cp in range(NCP):
                        _, kts, vas = cur
                        if cp + 1 < NCP:
                            nxt = make_kv_thunks2(cp + 1)
                        else:
                            nxt0 = make_kv_thunks2(0)
                            nxt = (phase_q_thunks(qt2_next) + nxt0[0],
                                   nxt0[1], nxt0[2])
                        n_thunks = len(nxt[0])
                        ti = 0
                        pair_idx = 0
                        for tch in range(NTCH):
                            for pb in range(4):
                                ps = psp.tile([128, 1024], f32, tag="ps")
                                nc.tensor.matmul(
                                    ps[:, 0:512],
                                    kts[0][0:64, pb * 128:(pb + 1) * 128],
                                    qt2[0:64, tch * 512:(tch + 1) * 512],
                                    start=True, stop=True)
                                nc.tensor.matmul(
                                    ps[:, 512:1024],
                                    kts[1][64:128, pb * 128:(pb + 1) * 128],
                                    qt2[64:128, tch * 512:(tch + 1) * 512],
                                    start=True, stop=True)
                                u = up.tile([128, 1024], u_dt, tag="u")
                                ei_acc += esplit
                                if ei_acc >= 1.0 - 1e-9:
                                    ei_acc -= 1.0
                                    nc.scalar.activation(u[:], ps[:], Exp,
                                                         scale=ACT_SCALE)
                                else:
                                    nc.vector.tensor_scalar(
                                        u[:].bitcast(u_bits), ps[:],
                                        DVE_BIAS, DVE_CLAMP,
                                        mybir.AluOpType.add,
                                        mybir.AluOpType.min)
                                prevs.append((u, vas, cp, tch, pb))
                                if len(prevs) > 2:
                                    emit_pv2(prevs.pop(0))
                                pair_idx += 1
                                target = min(n_thunks,
                                             (n_thunks * pair_idx) // 8)
                                while ti < target:
                                    nxt[0][ti]()
                                    ti += 1
                        while ti < n_thunks:
                            nxt[0][ti]()
                            ti += 1
                        cur = nxt
                    for p in prevs:
                        emit_pv2(p)
                    for tch in range(NTCH):
                        ot = otp.tile([128, 512], f32, tag="ot")
                        nc.vector.tensor_copy(ot[0:65, :], po[tch][0:65, :])
                        ob = obp.tile([128, 4 * 64], f32, tag="ob")
                        ob3 = ob[:].rearrange("p (j d) -> p j d", j=4)
                        for j in range(4):
                            pt = psp.tile([128, 65], f32, tag="ps",
                                          name="pt")
                            nc.tensor.matmul(pt[:],
                                             ot[0:65, j * 128:(j + 1) * 128],
                                             identf[0:65, 0:65],
                                             start=True, stop=True)
                            r = rp.tile([128, 1], f32, tag="r")
                            nc.vector.reciprocal(r[:], pt[:, 64:65])
                            nc.vector.tensor_scalar_mul(ob3[:, j, :],
                                                        pt[:, 0:64], r[:])
                        nc.sync.dma_start(out=out_v[tch], in_=ob3)
                    continue

                def make_kv_thunks(cp):
                    """Emit-later closures for loading/projecting s-chunk pair
                    cp.  Returns (thunks, kt_tiles, va_views).

                    All of one parity's PSUM traffic (kv projection + the 4
                    v-transposes) is packed into a single [128,1024] tile
                    (proj in bank cols 0:512, transposes at 512+65j), so a
                    pair costs 2 "ps" rotations instead of 10 and barely
                    disturbs the scores/exp double-buffer."""
                    kts, vas = [None, None], [None, None]
                    pw = [None, None]
                    thunks = []

                    def load(par):
                        def f():
                            sch = 2 * cp + par
                            enc = encp.tile([128, KT * 512], bf16, tag="enc")
                            enc3 = enc[:].rearrange("p (k n) -> p k n", k=KT)
                            nc.sync.dma_start(out=enc3, in_=encT_v[sch])
                            t1 = psp.tile([128, 1024], f32, tag="ps",
                                          name="pkv")
                            pkv = t1[:, 0:512]
                            wname = "kv" if par == 0 else "vk"
                            for k in range(KT):
                                nc.tensor.matmul(pkv, w_sb[wname][:, k, :],
                                                 enc3[:, k, :],
                                                 start=(k == 0),
                                                 stop=(k == KT - 1))
                            kt = ktp.tile([128, 512], bf16, tag="kt")
                            vt = vtp.tile([128, 512], bf16, tag="vt")
                            if par == 0:   # kT on rows 0:64, vT on rows 64:128
                                nc.vector.tensor_copy(kt[0:64, :], pkv[0:64, :])
                                nc.vector.tensor_copy(vt[64:128, :],
                                                      pkv[64:128, :])
                            else:
                                nc.vector.tensor_copy(kt[64:128, :],
                                                      pkv[64:128, :])
                                nc.vector.tensor_copy(vt[0:64, :], pkv[0:64, :])
                            va = vap.tile([128, 4 * 65], bf16, tag="va")
                            va3 = va[:].rearrange("p (j m) -> p j m", j=4)
                            nc.gpsimd.memset(va3[:, :, 64:65], 1.0)
                            kts[par] = kt
                            vas[par] = (va3, vt)
                            pw[par] = t1
                            # inline v-transposes: keeps t1's "ps"-rotation
                            # steal compact (one short window per parity)
                            rows = (slice(64, 128) if par == 0
                                    else slice(0, 64))
                            for j in range(4):
                                pvt = t1[:, 512 + 65 * j:512 + 65 * j + 65]
                                nc.tensor.matmul(
                                    pvt[:, 0:64],
                                    vt[rows, j * 128:(j + 1) * 128],
                                    ident[rows, rows],
                                    start=True, stop=True,
                                    skip_group_check=True)
                                nc.vector.tensor_copy(va3[:, j, 0:64],
                                                      pvt[:, 0:64])
                        return f

                    for par in range(2):
                        thunks.append(load(par))
                    return thunks, kts, vas

                def emit_pv(prev):
                    """PV matmuls for a previously-exp'd pair (one-pair SW
                    pipeline keeps the PE from stalling on the current exp)."""
                    u, pvas, pcp, ptch, ppb = prev
                    if pvlast and pcp != NCP - 1:
                        return
                    first = ((NCP - 1 if pvlast else 0) == pcp and ppb == 0)
                    last = (pcp == NCP - 1 and ppb == 3)
                    nc.tensor.matmul(po[ptch][0:65, :],
                                     pvas[0][:, ppb, :], u[:, 0:512],
                                     start=first, stop=False,
                                     skip_group_check=True)
                    nc.tensor.matmul(po[ptch][0:65, :],
                                     pvas[1][:, ppb, :], u[:, 512:1024],
                                     start=False, stop=last,
                                     skip_group_check=True)

                if cur is None:
                    cur = make_kv_thunks(0)
                    for th in cur[0]:
                        th()

                if wide:
                    # granule = one [128s x 512t] scores block, keyed
                    # (cp, tch, pb, par); 3 granules share one exp tile.
                    acc = [accp.tile([128, 512], f32, tag=f"acc{t}",
                                     name=f"acc{t}") for t in range(NTCH)]
                    open_ptv = {}

                    def tail_tch(tch):
                        """Normalize + store one t-chunk; emitted as soon as
                        its final PV spill lands so it overlaps the rest of
                        the exp stream instead of trailing the rep."""
                        ob = obp.tile([128, 4 * 64], f32, tag="ob")
                        ob3 = ob[:].rearrange("p (j d) -> p j d", j=4)
                        for j in range(4):
                            pt = psp.tile([128, 65], f32, tag="ps",
                                          name="pt")
                            nc.tensor.matmul(
                                pt[:],
                                acc[tch][0:65, j * 128:(j + 1) * 128],
                                identf[0:65, 0:65],
                                start=True, stop=True)
                            rr = rp.tile([128, 1], f32, tag="r")
                            nc.vector.reciprocal(rr[:], pt[:, 64:65])
                            nc.vector.tensor_scalar_mul(ob3[:, j, :],
                                                        pt[:, 0:64], rr[:])
                        nc.sync.dma_start(out=out_v[tch], in_=ob3)

                    def emit_pv_tile(entry):
                        u, gr_list = entry
                        for j, (gcp, gtch, gpb, gpar, gva) in \
                                enumerate(gr_list):
                            first = (gpb == 0 and gpar == 0)
                            if first:
                                open_ptv[(gcp, gtch)] = ptvp.tile(
                                    [128, 512], f32, tag="ptv",
                                    name="ptv")
                            ptv = open_ptv[(gcp, gtch)]
                            stop = (gpb == 3 and gpar == 1)
                            nc.tensor.matmul(
                                ptv[0:65, :], gva[:, gpb, :],
                                u[:, 512 * j:512 * (j + 1)],
                                start=first, stop=stop,
                                skip_group_check=True)
                            if stop:
                                del open_ptv[(gcp, gtch)]
                                if gcp == 0:
                                    nc.vector.tensor_copy(
                                        acc[gtch][0:65, :], ptv[0:65, :])
                                else:
                                    nc.vector.tensor_add(
                                        acc[gtch][0:65, :],
                                        acc[gtch][0:65, :], ptv[0:65, :])

                    GW = 3
                    n_gr = NCP * NTCH * 8
                    pend = []
                    tile_ps = None
                    gr_list = []
                    kts = vas = None
                    nxt = None
                    n_thunks = ti = 0
                    for gi in range(n_gr):
                        cp, w = divmod(gi, 32)
                        tch, r = divmod(w, 8)
                        pb, par = divmod(r, 2)
                        if w == 0:
                            _, kts, vas = cur
                            if cp + 1 < NCP:
                                nxt = make_kv_thunks(cp + 1)
                            else:
                                nxt0 = make_kv_thunks(0)
                                nxt = (phase_q_thunks(qt2_next) + nxt0[0],
                                       nxt0[1], nxt0[2])
                            n_thunks = len(nxt[0])
                            ti = 0
                        if tile_ps is None:
                            tile_ps = psp.tile([128, GW * 512], f32,
                                               tag="ps")
                            gr_list = []
                        col = 512 * len(gr_list)
                        rows = slice(0, 64) if par == 0 else slice(64, 128)
                        nc.tensor.matmul(
                            tile_ps[:, col:col + 512],
                            kts[par][rows, pb * 128:(pb + 1) * 128],
                            qt2[rows, tch * 512:(tch + 1) * 512],
                            start=True, stop=True)
                        gr_list.append((cp, tch, pb, par, vas[par][0]))
                        if len(gr_list) == GW or gi == n_gr - 1:
                            wcols = 512 * len(gr_list)
                            u = up.tile([128, GW * 512], bf16, tag="u")
                            nc.scalar.activation(u[:, 0:wcols],
                                                 tile_ps[:, 0:wcols],
                                                 Exp, scale=SCALE)
                            pend.append((u, gr_list))
                            tile_ps = None
                            if len(pend) > (2 if deep else 1):
                                emit_pv_tile(pend.pop(0))
                        tgt = min(n_thunks,
                                  (n_thunks * (w + 1)) // (24 if deep else 12))
                        while ti < tgt:
                            nxt[0][ti]()
                            ti += 1
                        if w == 31:
                            while ti < n_thunks:
                                nxt[0][ti]()
                                ti += 1
                            cur = nxt
                    for entry in pend:
                        emit_pv_tile(entry)
                    # tail: normalize straight from the SBUF accumulators.
                    # (Emitting each tail inline right after its final PV
                    # spill was tried and measured ~5µs SLOWER: the 16 extra
                    # "ps"-pool steals land in the rep-end region that
                    # already interleaves next-rep Q and kv thunks.)
                    for tch in range(NTCH):
                        tail_tch(tch)
                    continue

                prevs = []      # depth-2 exp->PV pipeline: PV never waits exp
                for cp in range(NCP):
                    _, kts, vas = cur
                    if nokv:
                        nxt = ([], kts, vas)
                    elif cp + 1 < NCP:
                        nxt = make_kv_thunks(cp + 1)
                    else:
                        # final pair: interleave next rep's Q projection and
                        # its s-pair 0 load instead of kv thunks
                        nxt0 = make_kv_thunks(0)
                        nxt = (phase_q_thunks(qt2_next) + nxt0[0],
                               nxt0[1], nxt0[2])
                    n_thunks = len(nxt[0])
                    ti = 0
                    pair_idx = 0
                    for tch in range(NTCH):
                        for pb in range(4):
                            ps = psp.tile([128, 1024], f32, tag="ps")
                            nc.tensor.matmul(
                                ps[:, 0:512],
                                kts[0][0:64, pb * 128:(pb + 1) * 128],
                                qt2[0:64, tch * 512:(tch + 1) * 512],
                                start=True, stop=True)
                            nc.tensor.matmul(
                                ps[:, 512:1024],
                                kts[1][64:128, pb * 128:(pb + 1) * 128],
                                qt2[64:128, tch * 512:(tch + 1) * 512],
                                start=True, stop=True)
                            if noexp:
                                u = u_ones
                            else:
                                u = up.tile([128, 1024], bf16, tag="u")
                                nc.scalar.activation(u[:], ps[:], Exp,
                                                     scale=SCALE)
                            prevs.append((u, (vas[0][0], vas[1][0]),
                                          cp, tch, pb))
                            if len(prevs) > 2:
                                emit_pv(prevs.pop(0))
                            pair_idx += 1
                            # interleave next chunk-pair's kv work over the
                            # first half of this pair (bunch: defer it all to
                            # the pair boundary)
                            target = (0 if bunch
                                      else min(n_thunks,
                                               (n_thunks * pair_idx) // 8))
                            while ti < target:
                                nxt[0][ti]()
                                ti += 1
                    while ti < n_thunks:
                        nxt[0][ti]()
                        ti += 1
                    cur = nxt
                for p in prevs:
                    emit_pv(p)

                # ---- Tail: normalize + store
                for tch in range(NTCH):
                    ot = otp.tile([128, 512], f32, tag="ot")
                    nc.vector.tensor_copy(ot[0:65, :], po[tch][0:65, :])
                    ob = obp.tile([128, 4 * 64], f32, tag="ob")
                    ob3 = ob[:].rearrange("p (j d) -> p j d", j=4)
                    for j in range(4):
                        pt = psp.tile([128, 65], f32, tag="ps", name="pt")
                        nc.tensor.matmul(pt[:],
                                         ot[0:65, j * 128:(j + 1) * 128],
                                         identf[0:65, 0:65],
                                         start=True, stop=True)
                        r = rp.tile([128, 1], f32, tag="r")
                        nc.vector.reciprocal(r[:], pt[:, 64:65])
                        nc.vector.tensor_scalar_mul(ob3[:, j, :],
                                                    pt[:, 0:64], r[:])
                    nc.sync.dma_start(out=out_v[tch], in_=ob3)

    nc.compile()
    return nc


def _get_nc(reps=1, **kw):
    key = (reps, tuple(sorted(kw.items())))
    if key not in _CACHE:
        _CACHE[key] = _build(reps, **kw)
    return _CACHE[key]


def _bf16(a):
    import ml_dtypes
    return np.asarray(a).astype(ml_dtypes.bfloat16)


def _pack_act(a, nch):
    """[L, C] row-major -> [L/512, 128, KT, 512] (chunk, partition, k, col)."""
    return np.ascontiguousarray(
        a.reshape(nch, 512, KT, 128).transpose(0, 3, 2, 1))


def _pack_w(w2):
    """[C, 128] -> [128, KT, 128]."""
    return np.ascontiguousarray(w2.reshape(KT, 128, 128).transpose(1, 0, 2))


def _prep_inputs(x, encode_out, Wq, Wk, Wv):
    x = _bf16(x)
    encode_out = _bf16(encode_out)
    Wq = _bf16(Wq)
    Wk = _bf16(Wk)
    Wv = _bf16(Wv)
    wqq = _pack_w(np.concatenate([Wq, Wq], axis=1))
    wkv = _pack_w(np.concatenate([Wk, Wv], axis=1))
    wvk = _pack_w(np.concatenate([Wv, Wk], axis=1))
    encTs = [_pack_act(encode_out[b], S // 512) for b in range(B)]
    in_maps = []
    for core in range(NCORE):
        b, th = divmod(core, 2)
        xTi = _pack_act(x[b, th * TSH:(th + 1) * TSH, :], NTCH)
        in_maps.append({"xT": xTi, "encT": encTs[b],
                        "Wqq": wqq, "Wkv": wkv, "Wvk": wvk})
    return in_maps


def kernel(x, encode_out, Wq, Wk, Wv):
    from concourse.bass_utils import run_bass_kernel_spmd
    nc = _get_nc(1, **BEST_KW)
    in_maps = _prep_inputs(x, encode_out, Wq, Wk, Wv)
    res = run_bass_kernel_spmd(nc, in_maps, list(range(NCORE)))
    out = np.empty((B, T, HS), dtype=np.float32)
    for core in range(NCORE):
        b, th = divmod(core, 2)
        o = res.results[core]["out"]            # [4, 128, 4, 64]
        out[b, th * TSH:(th + 1) * TSH] = (
            o.transpose(0, 2, 1, 3).reshape(TSH, HS))
    return out



# revision 13
# speedup vs baseline: 3.1409x; 1.3745x over previous
"""Cross-attention head (B=4, T=S=4096, C=1024, HS=64) on 8 TRN2 NeuronCores.

Sharding: core i handles batch b = i//2, query-half th = i%2 (2048 query rows).
Each core gets a transposed slice xT [C, 2048] and its batch's encT [C, S]
(host-side layout prep), plus packed weights Wqq=[Wq|Wq], Wkv=[Wk|Wv],
Wvk=[Wv|Wk] ([C,128] each).

All activations/weights are cast to bf16 on the host: halves HBM->SBUF DMA
traffic and runs every matmul at 1 cycle/row (fp32 needs 4).  PSUM
accumulation and the softmax normalization stay fp32; output is fp32.

Per-core pipeline:
  qT2 [128, 2048] = (Wqq)^T @ xT           rows 0:64 = q^T, rows 64:128 = copy
  stream over s in 512-chunks, alternating Wkv / Wvk so that k^T lands on
  partitions 0:64 (even chunks) or 64:128 (odd chunks); v^T on the other half.
  v^T chunks are transposed on the PE (identity matmul) into v_aug [128s, 65]
  tiles (col 64 = 1.0, giving the softmax denominator for free).
  scoresT [s,t] = kT^T_block @ qT2: two row-packed matmuls (tile rows 0:63 and
  64:127) run concurrently; one ACT Exp (scale=1/8) evacuates both PSUM banks
  to a bf16 U tile.
  PV: po[65, t] += v_aug^T @ U accumulated over all 32 s-blocks per t-chunk.
  Tail: po -> transpose -> divide rows by Z (col 64) -> out [2048, 64] fp32.
"""

import numpy as np

B, T, S, C, HS = 4, 4096, 4096, 1024, 64
NCORE = 8
TSH = T // 2            # 2048 query rows per core
KT = C // 128           # 8 contraction k-tiles
NTCH = TSH // 512       # 4 t-chunks
NCP = S // 1024         # 4 s-chunk pairs (each pair = 2x 512 keys)
SCALE = HS ** -0.5

_CACHE = {}

# build options for the shipped kernel (see _build); test.py reuses this
# so its timing measures the same configuration kernel() runs.
BEST_KW = {"wide": True}


def _build(reps=1, bunch=False, noexp=False, nokv=False, pvlast=False,
           wide=False, deep=False, pqps=False, esplit=None, pvdr=False,
           nodma=False):
    # esplit (new narrow path): fraction of exp tiles computed on ACT; the
    # rest use a DVE fast-exp bit trick (round(x*k+b) as uint viewed as
    # bf16/fp8 -- the Schraudolph approximation, ~1% rms weight error).
    # pvdr: U and va in fp8e4 and the two PV matmuls of a pair fused into
    # one DoubleRow matmul (2x PE throughput on the PV stream).
    # deep (wide mode only): exp->PV lag of 2 tiles instead of 1 and kv
    # thunks spread across the whole pair instead of the first 12 granules.
    # wide: exp tiles of [128, 3*512] (43 ACT instructions instead of 64,
    # cutting the per-instruction ACT tax) — enabled by spilling each
    # (s-pair, t-chunk) PV partial sum from a 2-bank transient PSUM
    # rotation into SBUF accumulators via DVE adds, freeing the 4 standing
    # po banks.  Mutually exclusive with the attribution variants.
    # noexp/nokv/pvlast are TIMING-ONLY attribution variants (wrong
    # results): noexp replaces the softmax exp with a constant-ones U tile
    # (removes ACT from the pipeline); nokv reuses s-chunk pair 0's k/v for
    # every pair (removes enc DMA + kv projection beyond the first pair);
    # pvlast emits PV matmuls only for the final s-chunk pair (removes 3/4
    # of the PV load from the PE).
    import concourse.bass as bass
    import concourse.mybir as mybir
    from concourse import bacc
    from concourse.tile import TileContext
    from concourse.masks import make_identity

    import math

    f32 = mybir.dt.float32
    bf16 = mybir.dt.bfloat16
    Exp = mybir.ActivationFunctionType.Exp

    newpath = esplit is not None
    if pvdr:
        assert newpath, "pvdr requires esplit"
    if newpath:
        assert not wide
        # qt2 is pre-scaled by C_PRE so the scores PSUM value is already in
        # "fast-exp units": u = psum + 127*128 (bf16) / 7*8 (fp8e4) rounds to
        # the bit pattern of ~exp(logit).  ACT granules undo the scaling via
        # the activation's free affine (scale=ln2/EBITS).
        EBITS = 8.0 if pvdr else 128.0
        C_PRE = SCALE * EBITS / math.log(2.0)
        ACT_SCALE = math.log(2.0) / EBITS
        # bias shifted down to center the Schraudolph chord error (mean-zero
        # relative error; otherwise ACT-exact and DVE-fastexp tiles in the
        # same softmax row disagree systematically by ~+4%)
        DVE_BIAS = (7.0 * 8.0 - 0.45) if pvdr else (127.0 * 128.0 - 7.2)
        DVE_CLAMP = 127.0 if pvdr else 32767.0
        u_dt = mybir.dt.float8e4 if pvdr else bf16
        u_bits = mybir.dt.uint8 if pvdr else mybir.dt.uint16

    nc = bacc.Bacc("TRN2", target_bir_lowering=False, debug=False,
                   num_devices=NCORE)
    # Host-packed layouts: every DMA sees per-partition contiguous runs.
    # xT packed as [tch, p, k, 512], encT as [sch, p, k, 512],
    # weights as [p, k, 128], out as [tch, p, j, 64].
    xT = nc.dram_tensor("xT", [NTCH, 128, KT, 512], bf16, kind="ExternalInput")
    encT = nc.dram_tensor("encT", [S // 512, 128, KT, 512], bf16,
                          kind="ExternalInput")
    wqq = nc.dram_tensor("Wqq", [128, KT, 128], bf16, kind="ExternalInput")
    wkv = nc.dram_tensor("Wkv", [128, KT, 128], bf16, kind="ExternalInput")
    wvk = nc.dram_tensor("Wvk", [128, KT, 128], bf16, kind="ExternalInput")
    out = nc.dram_tensor("out", [NTCH, 128, 4, HS], f32,
                         kind="ExternalOutput")

    xT_v = xT[:]       # [4, 128, 8, 512]
    encT_v = encT[:]   # [8, 128, 8, 512]
    out_v = out[:]     # [4, 128, 4, 64]

    with TileContext(nc) as tc:
        from contextlib import ExitStack
        with ExitStack() as ctx:
            ep = ctx.enter_context
            wpool = ep(tc.tile_pool(name="w", bufs=1))
            qpool = ep(tc.tile_pool(name="qt", bufs=2))
            xtp = ep(tc.tile_pool(name="xt", bufs=3))
            encp = ep(tc.tile_pool(name="enc", bufs=6))
            ktp = ep(tc.tile_pool(name="kt", bufs=4))
            vtp = ep(tc.tile_pool(name="vt", bufs=2))
            vap = ep(tc.tile_pool(name="va", bufs=4))
            up = ep(tc.tile_pool(name="u", bufs=5 if deep else 4))
            otp = ep(tc.tile_pool(name="ot", bufs=2))
            obp = ep(tc.tile_pool(name="ob", bufs=2))
            rp = ep(tc.tile_pool(name="r", bufs=2))
            # PSUM narrow: po 4 banks + shared pool 2x[128,1024] = 4 -> 8
            # PSUM wide: ptv 2x[128,512] = 2 + shared 2x[128,1536] = 6 -> 8
            if wide:
                accp = ep(tc.tile_pool(name="acc", bufs=2))
                ptvp = ep(tc.tile_pool(name="ptv", bufs=2, space="PSUM"))
            else:
                pop = ep(tc.tile_pool(name="po", bufs=1, space="PSUM"))
            psp = ep(tc.tile_pool(name="ps", bufs=2, space="PSUM"))

            # static tiles
            ident = wpool.tile([128, 128], bf16, tag="ident")
            make_identity(nc, ident[:])
            u_ones = None
            u_ones2 = None
            if noexp and not newpath:
                u_ones = wpool.tile([128, 1024], bf16, tag="u1")
                nc.gpsimd.memset(u_ones[:], 1.0)
            if noexp and newpath:
                u_ones2 = wpool.tile([128, 1024], u_dt, tag="u1")
                nc.gpsimd.memset(u_ones2[:], 1.0)
            identf = wpool.tile([128, 128], f32, tag="identf")
            make_identity(nc, identf[:])
            w_sb = {}
            for name, dram in (("qq", wqq), ("kv", wkv), ("vk", wvk)):
                wt = wpool.tile([128, KT * 128], bf16, tag=f"w{name}")
                nc.sync.dma_start(
                    out=wt[:].rearrange("p (k m) -> p k m", k=KT),
                    in_=dram[:])
                w_sb[name] = wt[:].rearrange("p (k m) -> p k m", k=KT)

            def phase_q_thunks(qt2):
                """Per-t-chunk thunks computing qT2 = [Wq|Wq]^T @ xT.
                Interleaved into the previous rep's final s-pair (which has
                no kv thunks), hiding the Q projection under the exp
                stream."""
                def load_x(tch):
                    def f():
                        xt = xtp.tile([128, KT * 512], bf16, tag="xt")
                        xt3 = xt[:].rearrange("p (k n) -> p k n", k=KT)
                        if not nodma:
                            nc.sync.dma_start(out=xt3, in_=xT_v[tch])
                        # wide mode: take pq from the ptv pool (its PV
                        # rotation has ~4µs slack/group) so the final pair
                        # carries the same 2 "ps"-pool steals as the others
                        if wide and not pqps:
                            pq = ptvp.tile([128, 512], f32, tag="ptv",
                                           name="pq")
                        else:
                            pq = psp.tile([128, 512], f32, tag="ps",
                                          name="pq")
                        for k in range(KT):
                            nc.tensor.matmul(pq[:], w_sb["qq"][:, k, :],
                                             xt3[:, k, :],
                                             start=(k == 0),
                                             stop=(k == KT - 1))
                        if newpath:
                            nc.vector.tensor_scalar_mul(
                                qt2[:, tch * 512:(tch + 1) * 512], pq[:],
                                C_PRE)
                        else:
                            nc.vector.tensor_copy(
                                qt2[:, tch * 512:(tch + 1) * 512], pq[:])
                    return f
                return [load_x(tch) for tch in range(NTCH)]

            qt2_next = None
            cur = None          # s-pair-0 kv state, pipelined across reps
            for _rep in range(reps):
                if qt2_next is None:
                    qt2 = qpool.tile([128, TSH], bf16, tag="qt2")
                    for th in phase_q_thunks(qt2):
                        th()
                else:
                    qt2 = qt2_next
                qt2_next = qpool.tile([128, TSH], bf16, tag="qt2")

                # ---- Phase S: stream s-chunk pairs
                po = None
                if not wide:
                    po = [pop.tile([128, 512], f32, tag=f"po{t}",
                                   name=f"po{t}")
                          for t in range(NTCH)]

                def make_kv_thunks2(cp):
                    """New-path kv thunks: one merged [128,512] kvt copy per
                    s-chunk (k and v halves stay in their PSUM rows) and one
                    merged strided va copy per parity.  pvdr: both parities'
                    v^T blocks land in a single fp8 va tile laid out
                    [128, (pb,par), 80] so a DR matmul reads ko=par pairs."""
                    kts = [None, None]
                    vas = [None, None]
                    thunks = []

                    def load(par):
                        def f():
                            sch = 2 * cp + par
                            enc = encp.tile([128, KT * 512], bf16, tag="enc")
                            enc3 = enc[:].rearrange("p (k n) -> p k n", k=KT)
                            if not nodma:
                                nc.sync.dma_start(out=enc3, in_=encT_v[sch])
                            t1 = psp.tile([128, 1024], f32, tag="ps",
                                          name="pkv")
                            pkv = t1[:, 0:512]
                            wname = "kv" if par == 0 else "vk"
                            for k in range(KT):
                                nc.tensor.matmul(pkv, w_sb[wname][:, k, :],
                                                 enc3[:, k, :],
                                                 start=(k == 0),
                                                 stop=(k == KT - 1))
                            kvt = ktp.tile([128, 512], bf16, tag="kt")
                            nc.vector.tensor_copy(kvt[:], pkv)
                            kts[par] = kvt
                            rows = (slice(64, 128) if par == 0
                                    else slice(0, 64))
                            for j in range(4):
                                pvt = t1[:, 512 + 65 * j:512 + 65 * j + 65]
                                nc.tensor.matmul(
                                    pvt[:, 0:64],
                                    kvt[rows, j * 128:(j + 1) * 128],
                                    ident[rows, rows],
                                    start=True, stop=True,
                                    skip_group_check=True)
                            src = t1[:, 512:512 + 4 * 65].rearrange(
                                "p (j m) -> p j m", j=4)[:, :, 0:64]
                            if pvdr:
                                if par == 0:
                                    va = vap.tile([128, 8 * 80], u_dt,
                                                  tag="va")
                                    va3 = va[:].rearrange(
                                        "p (j m) -> p j m", j=8)
                                    nc.gpsimd.memset(va3[:, :, 64:65], 1.0)
                                    vas[0] = vas[1] = va3
                                va3 = vas[0]
                                va4 = va3.rearrange(
                                    "p (j q) m -> p j q m", q=2)
                                nc.vector.tensor_copy(
                                    va4[:, :, par, 0:64], src)
                            else:
                                va = vap.tile([128, 4 * 65], bf16, tag="va")
                                va3 = va[:].rearrange("p (j m) -> p j m", j=4)
                                nc.gpsimd.memset(va3[:, :, 64:65], 1.0)
                                nc.vector.tensor_copy(va3[:, :, 0:64], src)
                                vas[par] = va3
                        return f

                    for par in range(2):
                        thunks.append(load(par))
                    return thunks, kts, vas

                def emit_pv2(prev):
                    u, pvas, pcp, ptch, ppb = prev
                    if pvlast and pcp != NCP - 1:
                        return
                    first = ((NCP - 1 if pvlast else 0) == pcp and ppb == 0)
                    last = (pcp == NCP - 1 and ppb == 3)
                    if pvdr:
                        u3 = u[:].rearrange("p (k n) -> p k n", k=2)
                        nc.tensor.matmul(
                            po[ptch][0:65, :],
                            pvas[0][:, 2 * ppb:2 * ppb + 2, 0:65], u3,
                            start=first, stop=last,
                            perf_mode=mybir.MatmulPerfMode.DoubleRow,
                            skip_group_check=True)
                    else:
                        nc.tensor.matmul(po[ptch][0:65, :],
                                         pvas[0][:, ppb, :], u[:, 0:512],
                                         start=first, stop=False,
                                         skip_group_check=True)
                        nc.tensor.matmul(po[ptch][0:65, :],
                                         pvas[1][:, ppb, :], u[:, 512:1024],
                                         start=False, stop=last,
                                         skip_group_check=True)

                if newpath:
                    if cur is None:
                        cur = make_kv_thunks2(0)
                        for th in cur[0]:
                            th()
                    prevs = []
                    ei_acc = 0.0
                    for cp in range(NCP):
                        _, kts, vas = cur
                        if nokv:
                            nxt = (([] if cp + 1 < NCP
                                    else phase_q_thunks(qt2_next)), kts, vas)
                        elif cp + 1 < NCP:
                            nxt = make_kv_thunks2(cp + 1)
                        else:
                            nxt0 = make_kv_thunks2(0)
                            nxt = (phase_q_thunks(qt2_next) + nxt0[0],
                                   nxt0[1], nxt0[2])
                        n_thunks = len(nxt[0])
                        ti = 0
                        pair_idx = 0
                        for tch in range(NTCH):
                            for pb in range(4):
                                ps = psp.tile([128, 1024], f32, tag="ps")
                                nc.tensor.matmul(
                                    ps[:, 0:512],
                                    kts[0][0:64, pb * 128:(pb + 1) * 128],
                                    qt2[0:64, tch * 512:(tch + 1) * 512],
                                    start=True, stop=True)
                                nc.tensor.matmul(
                                    ps[:, 512:1024],
                                    kts[1][64:128, pb * 128:(pb + 1) * 128],
                                    qt2[64:128, tch * 512:(tch + 1) * 512],
                                    start=True, stop=True)
                                if noexp:
                                    u = u_ones2
                                else:
                                    u = up.tile([128, 1024], u_dt, tag="u")
                                    ei_acc += esplit
                                    if ei_acc >= 1.0 - 1e-9:
                                        ei_acc -= 1.0
                                        nc.scalar.activation(u[:], ps[:], Exp,
                                                             scale=ACT_SCALE)
                                    else:
                                        nc.vector.tensor_scalar(
                                            u[:].bitcast(u_bits), ps[:],
                                            DVE_BIAS, DVE_CLAMP,
                                            mybir.AluOpType.add,
                                            mybir.AluOpType.min)
                                prevs.append((u, vas, cp, tch, pb))
                                if len(prevs) > 2:
                                    emit_pv2(prevs.pop(0))
                                pair_idx += 1
                                target = min(n_thunks,
                                             (n_thunks * pair_idx) // 8)
                                while ti < target:
                                    nxt[0][ti]()
                                    ti += 1
                        while ti < n_thunks:
                            nxt[0][ti]()
                            ti += 1
                        cur = nxt
                    for p in prevs:
                        emit_pv2(p)
                    for tch in range(NTCH):
                        ot = otp.tile([128, 512], f32, tag="ot")
                        nc.vector.tensor_copy(ot[0:65, :], po[tch][0:65, :])
                        ob = obp.tile([128, 4 * 64], f32, tag="ob")
                        ob3 = ob[:].rearrange("p (j d) -> p j d", j=4)
                        for j in range(4):
                            pt = psp.tile([128, 65], f32, tag="ps",
                                          name="pt")
                            nc.tensor.matmul(pt[:],
                                             ot[0:65, j * 128:(j + 1) * 128],
                                             identf[0:65, 0:65],
                                             start=True, stop=True)
                            r = rp.tile([128, 1], f32, tag="r")
                            nc.vector.reciprocal(r[:], pt[:, 64:65])
                            nc.vector.tensor_scalar_mul(ob3[:, j, :],
                                                        pt[:, 0:64], r[:])
                        nc.sync.dma_start(out=out_v[tch], in_=ob3)
                    continue

                def make_kv_thunks(cp):
                    """Emit-later closures for loading/projecting s-chunk pair
                    cp.  Returns (thunks, kt_tiles, va_views).

                    All of one parity's PSUM traffic (kv projection + the 4
                    v-transposes) is packed into a single [128,1024] tile
                    (proj in bank cols 0:512, transposes at 512+65j), so a
                    pair costs 2 "ps" rotations instead of 10 and barely
                    disturbs the scores/exp double-buffer."""
                    kts, vas = [None, None], [None, None]
                    pw = [None, None]
                    thunks = []

                    def load(par):
                        def f():
                            sch = 2 * cp + par
                            enc = encp.tile([128, KT * 512], bf16, tag="enc")
                            enc3 = enc[:].rearrange("p (k n) -> p k n", k=KT)
                            nc.sync.dma_start(out=enc3, in_=encT_v[sch])
                            t1 = psp.tile([128, 1024], f32, tag="ps",
                                          name="pkv")
                            pkv = t1[:, 0:512]
                            wname = "kv" if par == 0 else "vk"
                            for k in range(KT):
                                nc.tensor.matmul(pkv, w_sb[wname][:, k, :],
                                                 enc3[:, k, :],
                                                 start=(k == 0),
                                                 stop=(k == KT - 1))
                            kt = ktp.tile([128, 512], bf16, tag="kt")
                            vt = vtp.tile([128, 512], bf16, tag="vt")
                            if par == 0:   # kT on rows 0:64, vT on rows 64:128
                                nc.vector.tensor_copy(kt[0:64, :], pkv[0:64, :])
                                nc.vector.tensor_copy(vt[64:128, :],
                                                      pkv[64:128, :])
                            else:
                                nc.vector.tensor_copy(kt[64:128, :],
                                                      pkv[64:128, :])
                                nc.vector.tensor_copy(vt[0:64, :], pkv[0:64, :])
                            va = vap.tile([128, 4 * 65], bf16, tag="va")
                            va3 = va[:].rearrange("p (j m) -> p j m", j=4)
                            nc.gpsimd.memset(va3[:, :, 64:65], 1.0)
                            kts[par] = kt
                            vas[par] = (va3, vt)
                            pw[par] = t1
                            # inline v-transposes: keeps t1's "ps"-rotation
                            # steal compact (one short window per parity)
                            rows = (slice(64, 128) if par == 0
                                    else slice(0, 64))
                            for j in range(4):
                                pvt = t1[:, 512 + 65 * j:512 + 65 * j + 65]
                                nc.tensor.matmul(
                                    pvt[:, 0:64],
                                    vt[rows, j * 128:(j + 1) * 128],
                                    ident[rows, rows],
                                    start=True, stop=True,
                                    skip_group_check=True)
                                nc.vector.tensor_copy(va3[:, j, 0:64],
                                                      pvt[:, 0:64])
                        return f

                    for par in range(2):
                        thunks.append(load(par))
                    return thunks, kts, vas

                def emit_pv(prev):
                    """PV matmuls for a previously-exp'd pair (one-pair SW
                    pipeline keeps the PE from stalling on the current exp)."""
                    u, pvas, pcp, ptch, ppb = prev
                    if pvlast and pcp != NCP - 1:
                        return
                    first = ((NCP - 1 if pvlast else 0) == pcp and ppb == 0)
                    last = (pcp == NCP - 1 and ppb == 3)
                    nc.tensor.matmul(po[ptch][0:65, :],
                                     pvas[0][:, ppb, :], u[:, 0:512],
                                     start=first, stop=False,
                                     skip_group_check=True)
                    nc.tensor.matmul(po[ptch][0:65, :],
                                     pvas[1][:, ppb, :], u[:, 512:1024],
                                     start=False, stop=last,
                                     skip_group_check=True)

                if cur is None:
                    cur = make_kv_thunks(0)
                    for th in cur[0]:
                        th()

                if wide:
                    # granule = one [128s x 512t] scores block, keyed
                    # (cp, tch, pb, par); 3 granules share one exp tile.
                    acc = [accp.tile([128, 512], f32, tag=f"acc{t}",
                                     name=f"acc{t}") for t in range(NTCH)]
                    open_ptv = {}

                    def tail_tch(tch):
                        """Normalize + store one t-chunk; emitted as soon as
                        its final PV spill lands so it overlaps the rest of
                        the exp stream instead of trailing the rep."""
                        ob = obp.tile([128, 4 * 64], f32, tag="ob")
                        ob3 = ob[:].rearrange("p (j d) -> p j d", j=4)
                        for j in range(4):
                            pt = psp.tile([128, 65], f32, tag="ps",
                                          name="pt")
                            nc.tensor.matmul(
                                pt[:],
                                acc[tch][0:65, j * 128:(j + 1) * 128],
                                identf[0:65, 0:65],
                                start=True, stop=True)
                            rr = rp.tile([128, 1], f32, tag="r")
                            nc.vector.reciprocal(rr[:], pt[:, 64:65])
                            nc.vector.tensor_scalar_mul(ob3[:, j, :],
                                                        pt[:, 0:64], rr[:])
                        nc.sync.dma_start(out=out_v[tch], in_=ob3)

                    def emit_pv_tile(entry):
                        u, gr_list = entry
                        for j, (gcp, gtch, gpb, gpar, gva) in \
                                enumerate(gr_list):
                            first = (gpb == 0 and gpar == 0)
                            if first:
                                open_ptv[(gcp, gtch)] = ptvp.tile(
                                    [128, 512], f32, tag="ptv",
                                    name="ptv")
                            ptv = open_ptv[(gcp, gtch)]
                            stop = (gpb == 3 and gpar == 1)
                            nc.tensor.matmul(
                                ptv[0:65, :], gva[:, gpb, :],
                                u[:, 512 * j:512 * (j + 1)],
                                start=first, stop=stop,
                                skip_group_check=True)
                            if stop:
                                del open_ptv[(gcp, gtch)]
                                if gcp == 0:
                                    nc.vector.tensor_copy(
                                        acc[gtch][0:65, :], ptv[0:65, :])
                                else:
                                    nc.vector.tensor_add(
                                        acc[gtch][0:65, :],
                                        acc[gtch][0:65, :], ptv[0:65, :])

                    GW = 3
                    n_gr = NCP * NTCH * 8
                    pend = []
                    tile_ps = None
                    gr_list = []
                    kts = vas = None
                    nxt = None
                    n_thunks = ti = 0
                    for gi in range(n_gr):
                        cp, w = divmod(gi, 32)
                        tch, r = divmod(w, 8)
                        pb, par = divmod(r, 2)
                        if w == 0:
                            _, kts, vas = cur
                            if cp + 1 < NCP:
                                nxt = make_kv_thunks(cp + 1)
                            else:
                                nxt0 = make_kv_thunks(0)
                                nxt = (phase_q_thunks(qt2_next) + nxt0[0],
                                       nxt0[1], nxt0[2])
                            n_thunks = len(nxt[0])
                            ti = 0
                        if tile_ps is None:
                            tile_ps = psp.tile([128, GW * 512], f32,
                                               tag="ps")
                            gr_list = []
                        col = 512 * len(gr_list)
                        rows = slice(0, 64) if par == 0 else slice(64, 128)
                        nc.tensor.matmul(
                            tile_ps[:, col:col + 512],
                            kts[par][rows, pb * 128:(pb + 1) * 128],
                            qt2[rows, tch * 512:(tch + 1) * 512],
                            start=True, stop=True)
                        gr_list.append((cp, tch, pb, par, vas[par][0]))
                        if len(gr_list) == GW or gi == n_gr - 1:
                            wcols = 512 * len(gr_list)
                            u = up.tile([128, GW * 512], bf16, tag="u")
                            nc.scalar.activation(u[:, 0:wcols],
                                                 tile_ps[:, 0:wcols],
                                                 Exp, scale=SCALE)
                            pend.append((u, gr_list))
                            tile_ps = None
                            if len(pend) > (2 if deep else 1):
                                emit_pv_tile(pend.pop(0))
                        tgt = min(n_thunks,
                                  (n_thunks * (w + 1)) // (24 if deep else 12))
                        while ti < tgt:
                            nxt[0][ti]()
                            ti += 1
                        if w == 31:
                            while ti < n_thunks:
                                nxt[0][ti]()
                                ti += 1
                            cur = nxt
                    for entry in pend:
                        emit_pv_tile(entry)
                    # tail: normalize straight from the SBUF accumulators.
                    # (Emitting each tail inline right after its final PV
                    # spill was tried and measured ~5µs SLOWER: the 16 extra
                    # "ps"-pool steals land in the rep-end region that
                    # already interleaves next-rep Q and kv thunks.)
                    for tch in range(NTCH):
                        tail_tch(tch)
                    continue

                prevs = []      # depth-2 exp->PV pipeline: PV never waits exp
                for cp in range(NCP):
                    _, kts, vas = cur
                    if nokv:
                        nxt = ([], kts, vas)
                    elif cp + 1 < NCP:
                        nxt = make_kv_thunks(cp + 1)
                    else:
                        # final pair: interleave next rep's Q projection and
                        # its s-pair 0 load instead of kv thunks
                        nxt0 = make_kv_thunks(0)
                        nxt = (phase_q_thunks(qt2_next) + nxt0[0],
                               nxt0[1], nxt0[2])
                    n_thunks = len(nxt[0])
                    ti = 0
                    pair_idx = 0
                    for tch in range(NTCH):
                        for pb in range(4):
                            ps = psp.tile([128, 1024], f32, tag="ps")
                            nc.tensor.matmul(
                                ps[:, 0:512],
                                kts[0][0:64, pb * 128:(pb + 1) * 128],
                                qt2[0:64, tch * 512:(tch + 1) * 512],
                                start=True, stop=True)
                            nc.tensor.matmul(
                                ps[:, 512:1024],
                                kts[1][64:128, pb * 128:(pb + 1) * 128],
                                qt2[64:128, tch * 512:(tch + 1) * 512],
                                start=True, stop=True)
                            if noexp:
                                u = u_ones
                            else:
                                u = up.tile([128, 1024], bf16, tag="u")
                                nc.scalar.activation(u[:], ps[:], Exp,
                                                     scale=SCALE)
                            prevs.append((u, (vas[0][0], vas[1][0]),
                                          cp, tch, pb))
                            if len(prevs) > 2:
                                emit_pv(prevs.pop(0))
                            pair_idx += 1
                            # interleave next chunk-pair's kv work over the
                            # first half of this pair (bunch: defer it all to
                            # the pair boundary)
                            target = (0 if bunch
                                      else min(n_thunks,
                                               (n_thunks * pair_idx) // 8))
                            while ti < target:
                                nxt[0][ti]()
                                ti += 1
                    while ti < n_thunks:
                        nxt[0][ti]()
                        ti += 1
                    cur = nxt
                for p in prevs:
                    emit_pv(p)

                # ---- Tail: normalize + store
                for tch in range(NTCH):
                    ot = otp.tile([128, 512], f32, tag="ot")
                    nc.vector.tensor_copy(ot[0:65, :], po[tch][0:65, :])
                    ob = obp.tile([128, 4 * 64], f32, tag="ob")
                    ob3 = ob[:].rearrange("p (j d) -> p j d", j=4)
                    for j in range(4):
                        pt = psp.tile([128, 65], f32, tag="ps", name="pt")
                        nc.tensor.matmul(pt[:],
                                         ot[0:65, j * 128:(j + 1) * 128],
                                         identf[0:65, 0:65],
                                         start=True, stop=True)
                        r = rp.tile([128, 1], f32, tag="r")
                        nc.vector.reciprocal(r[:], pt[:, 64:65])
                        nc.vector.tensor_scalar_mul(ob3[:, j, :],
                                                    pt[:, 0:64], r[:])
                    nc.sync.dma_start(out=out_v[tch], in_=ob3)

    nc.compile()
    return nc


def _get_nc(reps=1, **kw):
    key = (reps, tuple(sorted(kw.items())))
    if key not in _CACHE:
        _CACHE[key] = _build(reps, **kw)
    return _CACHE[key]


def _bf16(a):
    import ml_dtypes
    return np.asarray(a).astype(ml_dtypes.bfloat16)


def _pack_act(a, nch):
    """[L, C] row-major -> [L/512, 128, KT, 512] (chunk, partition, k, col)."""
    return np.ascontiguousarray(
        a.reshape(nch, 512, KT, 128).transpose(0, 3, 2, 1))


def _pack_w(w2):
    """[C, 128] -> [128, KT, 128]."""
    return np.ascontiguousarray(w2.reshape(KT, 128, 128).transpose(1, 0, 2))


def _prep_inputs(x, encode_out, Wq, Wk, Wv):
    x = _bf16(x)
    encode_out = _bf16(encode_out)
    Wq = _bf16(Wq)
    Wk = _bf16(Wk)
    Wv = _bf16(Wv)
    wqq = _pack_w(np.concatenate([Wq, Wq], axis=1))
    wkv = _pack_w(np.concatenate([Wk, Wv], axis=1))
    wvk = _pack_w(np.concatenate([Wv, Wk], axis=1))
    encTs = [_pack_act(encode_out[b], S // 512) for b in range(B)]
    in_maps = []
    for core in range(NCORE):
        b, th = divmod(core, 2)
        xTi = _pack_act(x[b, th * TSH:(th + 1) * TSH, :], NTCH)
        in_maps.append({"xT": xTi, "encT": encTs[b],
                        "Wqq": wqq, "Wkv": wkv, "Wvk": wvk})
    return in_maps


def kernel(x, encode_out, Wq, Wk, Wv):
    from concourse.bass_utils import run_bass_kernel_spmd
    nc = _get_nc(1, **BEST_KW)
    in_maps = _prep_inputs(x, encode_out, Wq, Wk, Wv)
    res = run_bass_kernel_spmd(nc, in_maps, list(range(NCORE)))
    out = np.empty((B, T, HS), dtype=np.float32)
    for core in range(NCORE):
        b, th = divmod(core, 2)
        o = res.results[core]["out"]            # [4, 128, 4, 64]
        out[b, th * TSH:(th + 1) * TSH] = (
            o.transpose(0, 2, 1, 3).reshape(TSH, HS))
    return out



# revision 16
# speedup vs baseline: 3.2326x; 1.0292x over previous
"""Cross-attention head (B=4, T=S=4096, C=1024, HS=64) on 8 TRN2 NeuronCores.

Sharding: core i handles batch b = i//2, query-half th = i%2 (2048 query rows).
Each core gets a transposed slice xT [C, 2048] and its batch's encT [C, S]
(host-side layout prep), plus packed weights Wqq=[Wq|Wq], Wkv=[Wk|Wv],
Wvk=[Wv|Wk] ([C,128] each).

All activations/weights are cast to bf16 on the host: halves HBM->SBUF DMA
traffic and runs every matmul at 1 cycle/row (fp32 needs 4).  PSUM
accumulation and the softmax normalization stay fp32; output is fp32.

Per-core pipeline:
  qT2 [128, 2048] = (Wqq)^T @ xT           rows 0:64 = q^T, rows 64:128 = copy
  stream over s in 512-chunks, alternating Wkv / Wvk so that k^T lands on
  partitions 0:64 (even chunks) or 64:128 (odd chunks); v^T on the other half.
  v^T chunks are transposed on the PE (identity matmul) into v_aug [128s, 65]
  tiles (col 64 = 1.0, giving the softmax denominator for free).
  scoresT [s,t] = kT^T_block @ qT2: two row-packed matmuls (tile rows 0:63 and
  64:127) run concurrently; one ACT Exp (scale=1/8) evacuates both PSUM banks
  to a bf16 U tile.
  PV: po[65, t] += v_aug^T @ U accumulated over all 32 s-blocks per t-chunk.
  Tail: po -> transpose -> divide rows by Z (col 64) -> out [2048, 64] fp32.
"""

import numpy as np

B, T, S, C, HS = 4, 4096, 4096, 1024, 64
NCORE = 8
TSH = T // 2            # 2048 query rows per core
KT = C // 128           # 8 contraction k-tiles
NTCH = TSH // 512       # 4 t-chunks
NCP = S // 1024         # 4 s-chunk pairs (each pair = 2x 512 keys)
SCALE = HS ** -0.5

_CACHE = {}

# build options for the shipped kernel (see _build); test.py reuses this
# so its timing measures the same configuration kernel() runs.
BEST_KW = {"wide": True}


def _build(reps=1, bunch=False, noexp=False, nokv=False, pvlast=False,
           wide=False, deep=False, pqps=False, esplit=None, pvdr=False,
           nodma=False, tspread=8, ubufs=4):
    # esplit (new narrow path): fraction of exp tiles computed on ACT; the
    # rest use a DVE fast-exp bit trick (round(x*k+b) as uint viewed as
    # bf16/fp8 -- the Schraudolph approximation, ~1% rms weight error).
    # pvdr: U and va in fp8e4 and the two PV matmuls of a pair fused into
    # one DoubleRow matmul (2x PE throughput on the PV stream).
    # deep (wide mode only): exp->PV lag of 2 tiles instead of 1 and kv
    # thunks spread across the whole pair instead of the first 12 granules.
    # wide: exp tiles of [128, 3*512] (43 ACT instructions instead of 64,
    # cutting the per-instruction ACT tax) — enabled by spilling each
    # (s-pair, t-chunk) PV partial sum from a 2-bank transient PSUM
    # rotation into SBUF accumulators via DVE adds, freeing the 4 standing
    # po banks.  Mutually exclusive with the attribution variants.
    # noexp/nokv/pvlast are TIMING-ONLY attribution variants (wrong
    # results): noexp replaces the softmax exp with a constant-ones U tile
    # (removes ACT from the pipeline); nokv reuses s-chunk pair 0's k/v for
    # every pair (removes enc DMA + kv projection beyond the first pair);
    # pvlast emits PV matmuls only for the final s-chunk pair (removes 3/4
    # of the PV load from the PE).
    import concourse.bass as bass
    import concourse.mybir as mybir
    from concourse import bacc
    from concourse.tile import TileContext
    from concourse.masks import make_identity

    import math

    f32 = mybir.dt.float32
    bf16 = mybir.dt.bfloat16
    Exp = mybir.ActivationFunctionType.Exp

    newpath = esplit is not None
    if pvdr:
        assert newpath, "pvdr requires esplit"
    if newpath:
        assert not wide
        # qt2 is pre-scaled by C_PRE so the scores PSUM value is already in
        # "fast-exp units": u = psum + 127*128 (bf16) / 7*8 (fp8e4) rounds to
        # the bit pattern of ~exp(logit).  ACT granules undo the scaling via
        # the activation's free affine (scale=ln2/EBITS).
        EBITS = 8.0 if pvdr else 128.0
        C_PRE = SCALE * EBITS / math.log(2.0)
        ACT_SCALE = math.log(2.0) / EBITS
        # bias shifted down to center the Schraudolph chord error (mean-zero
        # relative error; otherwise ACT-exact and DVE-fastexp tiles in the
        # same softmax row disagree systematically by ~+4%)
        DVE_BIAS = (7.0 * 8.0 - 0.45) if pvdr else (127.0 * 128.0 - 7.2)
        DVE_CLAMP = 127.0 if pvdr else 32767.0
        u_dt = mybir.dt.float8e4 if pvdr else bf16
        u_bits = mybir.dt.uint8 if pvdr else mybir.dt.uint16

    nc = bacc.Bacc("TRN2", target_bir_lowering=False, debug=False,
                   num_devices=NCORE)
    # Host-packed layouts: every DMA sees per-partition contiguous runs.
    # xT packed as [tch, p, k, 512], encT as [sch, p, k, 512],
    # weights as [p, k, 128], out as [tch, p, j, 64].
    xT = nc.dram_tensor("xT", [NTCH, 128, KT, 512], bf16, kind="ExternalInput")
    encT = nc.dram_tensor("encT", [S // 512, 128, KT, 512], bf16,
                          kind="ExternalInput")
    wqq = nc.dram_tensor("Wqq", [128, KT, 128], bf16, kind="ExternalInput")
    wkv = nc.dram_tensor("Wkv", [128, KT, 128], bf16, kind="ExternalInput")
    wvk = nc.dram_tensor("Wvk", [128, KT, 128], bf16, kind="ExternalInput")
    out = nc.dram_tensor("out", [NTCH, 128, 4, HS], f32,
                         kind="ExternalOutput")

    xT_v = xT[:]       # [4, 128, 8, 512]
    encT_v = encT[:]   # [8, 128, 8, 512]
    out_v = out[:]     # [4, 128, 4, 64]

    with TileContext(nc) as tc:
        from contextlib import ExitStack
        with ExitStack() as ctx:
            ep = ctx.enter_context
            wpool = ep(tc.tile_pool(name="w", bufs=1))
            qpool = ep(tc.tile_pool(name="qt", bufs=2))
            xtp = ep(tc.tile_pool(name="xt", bufs=3))
            encp = ep(tc.tile_pool(name="enc", bufs=6))
            ktp = ep(tc.tile_pool(name="kt", bufs=4))
            vtp = ep(tc.tile_pool(name="vt", bufs=2))
            vap = ep(tc.tile_pool(name="va", bufs=4))
            up = ep(tc.tile_pool(name="u", bufs=5 if deep else ubufs))
            otp = ep(tc.tile_pool(name="ot", bufs=2))
            obp = ep(tc.tile_pool(name="ob", bufs=2))
            rp = ep(tc.tile_pool(name="r", bufs=2))
            # PSUM narrow: po 4 banks + shared pool 2x[128,1024] = 4 -> 8
            # PSUM wide: ptv 2x[128,512] = 2 + shared 2x[128,1536] = 6 -> 8
            if wide:
                accp = ep(tc.tile_pool(name="acc", bufs=2))
                ptvp = ep(tc.tile_pool(name="ptv", bufs=2, space="PSUM"))
            else:
                pop = ep(tc.tile_pool(name="po", bufs=1, space="PSUM"))
            psp = ep(tc.tile_pool(name="ps", bufs=2, space="PSUM"))

            # static tiles
            ident = wpool.tile([128, 128], bf16, tag="ident")
            make_identity(nc, ident[:])
            u_ones = None
            u_ones2 = None
            if noexp and not newpath:
                u_ones = wpool.tile([128, 1024], bf16, tag="u1")
                nc.gpsimd.memset(u_ones[:], 1.0)
            if noexp and newpath:
                u_ones2 = wpool.tile([128, 1024], u_dt, tag="u1")
                nc.gpsimd.memset(u_ones2[:], 1.0)
            identf = wpool.tile([128, 128], f32, tag="identf")
            make_identity(nc, identf[:])
            w_sb = {}
            for name, dram in (("qq", wqq), ("kv", wkv), ("vk", wvk)):
                wt = wpool.tile([128, KT * 128], bf16, tag=f"w{name}")
                nc.sync.dma_start(
                    out=wt[:].rearrange("p (k m) -> p k m", k=KT),
                    in_=dram[:])
                w_sb[name] = wt[:].rearrange("p (k m) -> p k m", k=KT)

            def phase_q_thunks(qt2):
                """Per-t-chunk thunks computing qT2 = [Wq|Wq]^T @ xT.
                Interleaved into the previous rep's final s-pair (which has
                no kv thunks), hiding the Q projection under the exp
                stream."""
                def load_x(tch):
                    def f():
                        xt = xtp.tile([128, KT * 512], bf16, tag="xt")
                        xt3 = xt[:].rearrange("p (k n) -> p k n", k=KT)
                        if not nodma:
                            nc.sync.dma_start(out=xt3, in_=xT_v[tch])
                        # wide mode: take pq from the ptv pool (its PV
                        # rotation has ~4µs slack/group) so the final pair
                        # carries the same 2 "ps"-pool steals as the others
                        if wide and not pqps:
                            pq = ptvp.tile([128, 512], f32, tag="ptv",
                                           name="pq")
                        else:
                            pq = psp.tile([128, 512], f32, tag="ps",
                                          name="pq")
                        for k in range(KT):
                            nc.tensor.matmul(pq[:], w_sb["qq"][:, k, :],
                                             xt3[:, k, :],
                                             start=(k == 0),
                                             stop=(k == KT - 1))
                        if newpath:
                            nc.vector.tensor_scalar_mul(
                                qt2[:, tch * 512:(tch + 1) * 512], pq[:],
                                C_PRE)
                        else:
                            nc.vector.tensor_copy(
                                qt2[:, tch * 512:(tch + 1) * 512], pq[:])
                    return f
                return [load_x(tch) for tch in range(NTCH)]

            qt2_next = None
            cur = None          # s-pair-0 kv state, pipelined across reps
            for _rep in range(reps):
                if qt2_next is None:
                    qt2 = qpool.tile([128, TSH], bf16, tag="qt2")
                    for th in phase_q_thunks(qt2):
                        th()
                else:
                    qt2 = qt2_next
                qt2_next = qpool.tile([128, TSH], bf16, tag="qt2")

                # ---- Phase S: stream s-chunk pairs
                po = None
                if not wide:
                    po = [pop.tile([128, 512], f32, tag=f"po{t}",
                                   name=f"po{t}")
                          for t in range(NTCH)]

                def make_kv_thunks2(cp):
                    """New-path kv thunks: one merged [128,512] kvt copy per
                    s-chunk (k and v halves stay in their PSUM rows) and one
                    merged strided va copy per parity.  pvdr: both parities'
                    v^T blocks land in a single fp8 va tile laid out
                    [128, (pb,par), 80] so a DR matmul reads ko=par pairs."""
                    kts = [None, None]
                    vas = [None, None]
                    thunks = []

                    def load(par):
                        def f():
                            sch = 2 * cp + par
                            enc = encp.tile([128, KT * 512], bf16, tag="enc")
                            enc3 = enc[:].rearrange("p (k n) -> p k n", k=KT)
                            if not nodma:
                                nc.sync.dma_start(out=enc3, in_=encT_v[sch])
                            t1 = psp.tile([128, 1024], f32, tag="ps",
                                          name="pkv")
                            pkv = t1[:, 0:512]
                            wname = "kv" if par == 0 else "vk"
                            for k in range(KT):
                                nc.tensor.matmul(pkv, w_sb[wname][:, k, :],
                                                 enc3[:, k, :],
                                                 start=(k == 0),
                                                 stop=(k == KT - 1))
                            kvt = ktp.tile([128, 512], bf16, tag="kt")
                            nc.vector.tensor_copy(kvt[:], pkv)
                            kts[par] = kvt
                            rows = (slice(64, 128) if par == 0
                                    else slice(0, 64))
                            for j in range(4):
                                pvt = t1[:, 512 + 65 * j:512 + 65 * j + 65]
                                nc.tensor.matmul(
                                    pvt[:, 0:64],
                                    kvt[rows, j * 128:(j + 1) * 128],
                                    ident[rows, rows],
                                    start=True, stop=True,
                                    skip_group_check=True)
                            src = t1[:, 512:512 + 4 * 65].rearrange(
                                "p (j m) -> p j m", j=4)[:, :, 0:64]
                            if pvdr:
                                if par == 0:
                                    va = vap.tile([128, 8 * 80], u_dt,
                                                  tag="va")
                                    va3 = va[:].rearrange(
                                        "p (j m) -> p j m", j=8)
                                    nc.gpsimd.memset(va3[:, :, 64:65], 1.0)
                                    vas[0] = vas[1] = va3
                                va3 = vas[0]
                                va4 = va3.rearrange(
                                    "p (j q) m -> p j q m", q=2)
                                nc.vector.tensor_copy(
                                    va4[:, :, par, 0:64], src)
                            else:
                                va = vap.tile([128, 4 * 65], bf16, tag="va")
                                va3 = va[:].rearrange("p (j m) -> p j m", j=4)
                                nc.gpsimd.memset(va3[:, :, 64:65], 1.0)
                                nc.vector.tensor_copy(va3[:, :, 0:64], src)
                                vas[par] = va3
                        return f

                    for par in range(2):
                        thunks.append(load(par))
                    return thunks, kts, vas

                def emit_pv2(prev):
                    u, pvas, pcp, ptch, ppb = prev
                    if pvlast and pcp != NCP - 1:
                        return
                    first = ((NCP - 1 if pvlast else 0) == pcp and ppb == 0)
                    last = (pcp == NCP - 1 and ppb == 3)
                    if pvdr:
                        u3 = u[:].rearrange("p (k n) -> p k n", k=2)
                        nc.tensor.matmul(
                            po[ptch][0:65, :],
                            pvas[0][:, 2 * ppb:2 * ppb + 2, 0:65], u3,
                            start=first, stop=last,
                            perf_mode=mybir.MatmulPerfMode.DoubleRow,
                            skip_group_check=True)
                    else:
                        nc.tensor.matmul(po[ptch][0:65, :],
                                         pvas[0][:, ppb, :], u[:, 0:512],
                                         start=first, stop=False,
                                         skip_group_check=True)
                        nc.tensor.matmul(po[ptch][0:65, :],
                                         pvas[1][:, ppb, :], u[:, 512:1024],
                                         start=False, stop=last,
                                         skip_group_check=True)

                if newpath:
                    if cur is None:
                        cur = make_kv_thunks2(0)
                        for th in cur[0]:
                            th()
                    prevs = []
                    ei_acc = 0.0
                    for cp in range(NCP):
                        _, kts, vas = cur
                        if nokv:
                            nxt = (([] if cp + 1 < NCP
                                    else phase_q_thunks(qt2_next)), kts, vas)
                        elif cp + 1 < NCP:
                            nxt = make_kv_thunks2(cp + 1)
                        else:
                            nxt0 = make_kv_thunks2(0)
                            nxt = (phase_q_thunks(qt2_next) + nxt0[0],
                                   nxt0[1], nxt0[2])
                        n_thunks = len(nxt[0])
                        ti = 0
                        pair_idx = 0
                        for tch in range(NTCH):
                            for pb in range(4):
                                ps = psp.tile([128, 1024], f32, tag="ps")
                                nc.tensor.matmul(
                                    ps[:, 0:512],
                                    kts[0][0:64, pb * 128:(pb + 1) * 128],
                                    qt2[0:64, tch * 512:(tch + 1) * 512],
                                    start=True, stop=True)
                                nc.tensor.matmul(
                                    ps[:, 512:1024],
                                    kts[1][64:128, pb * 128:(pb + 1) * 128],
                                    qt2[64:128, tch * 512:(tch + 1) * 512],
                                    start=True, stop=True)
                                if noexp:
                                    u = u_ones2
                                else:
                                    u = up.tile([128, 1024], u_dt, tag="u")
                                    ei_acc += esplit
                                    if ei_acc >= 1.0 - 1e-9:
                                        ei_acc -= 1.0
                                        nc.scalar.activation(u[:], ps[:], Exp,
                                                             scale=ACT_SCALE)
                                    else:
                                        nc.vector.tensor_scalar(
                                            u[:].bitcast(u_bits), ps[:],
                                            DVE_BIAS, DVE_CLAMP,
                                            mybir.AluOpType.add,
                                            mybir.AluOpType.min)
                                prevs.append((u, vas, cp, tch, pb))
                                if len(prevs) > 2:
                                    emit_pv2(prevs.pop(0))
                                pair_idx += 1
                                target = min(n_thunks,
                                             (n_thunks * pair_idx) // tspread)
                                while ti < target:
                                    nxt[0][ti]()
                                    ti += 1
                        while ti < n_thunks:
                            nxt[0][ti]()
                            ti += 1
                        cur = nxt
                    for p in prevs:
                        emit_pv2(p)
                    for tch in range(NTCH):
                        ot = otp.tile([128, 512], f32, tag="ot")
                        nc.vector.tensor_copy(ot[0:65, :], po[tch][0:65, :])
                        ob = obp.tile([128, 4 * 64], f32, tag="ob")
                        ob3 = ob[:].rearrange("p (j d) -> p j d", j=4)
                        for j in range(4):
                            pt = psp.tile([128, 65], f32, tag="ps",
                                          name="pt")
                            nc.tensor.matmul(pt[:],
                                             ot[0:65, j * 128:(j + 1) * 128],
                                             identf[0:65, 0:65],
                                             start=True, stop=True)
                            r = rp.tile([128, 1], f32, tag="r")
                            nc.vector.reciprocal(r[:], pt[:, 64:65])
                            nc.vector.tensor_scalar_mul(ob3[:, j, :],
                                                        pt[:, 0:64], r[:])
                        nc.sync.dma_start(out=out_v[tch], in_=ob3)
                    continue

                def make_kv_thunks(cp):
                    """Emit-later closures for loading/projecting s-chunk pair
                    cp.  Returns (thunks, kt_tiles, va_views).

                    All of one parity's PSUM traffic (kv projection + the 4
                    v-transposes) is packed into a single [128,1024] tile
                    (proj in bank cols 0:512, transposes at 512+65j), so a
                    pair costs 2 "ps" rotations instead of 10 and barely
                    disturbs the scores/exp double-buffer."""
                    kts, vas = [None, None], [None, None]
                    pw = [None, None]
                    thunks = []

                    def load(par):
                        def f():
                            sch = 2 * cp + par
                            enc = encp.tile([128, KT * 512], bf16, tag="enc")
                            enc3 = enc[:].rearrange("p (k n) -> p k n", k=KT)
                            nc.sync.dma_start(out=enc3, in_=encT_v[sch])
                            t1 = psp.tile([128, 1024], f32, tag="ps",
                                          name="pkv")
                            pkv = t1[:, 0:512]
                            wname = "kv" if par == 0 else "vk"
                            for k in range(KT):
                                nc.tensor.matmul(pkv, w_sb[wname][:, k, :],
                                                 enc3[:, k, :],
                                                 start=(k == 0),
                                                 stop=(k == KT - 1))
                            kt = ktp.tile([128, 512], bf16, tag="kt")
                            vt = vtp.tile([128, 512], bf16, tag="vt")
                            if par == 0:   # kT on rows 0:64, vT on rows 64:128
                                nc.vector.tensor_copy(kt[0:64, :], pkv[0:64, :])
                                nc.vector.tensor_copy(vt[64:128, :],
                                                      pkv[64:128, :])
                            else:
                                nc.vector.tensor_copy(kt[64:128, :],
                                                      pkv[64:128, :])
                                nc.vector.tensor_copy(vt[0:64, :], pkv[0:64, :])
                            va = vap.tile([128, 4 * 65], bf16, tag="va")
                            va3 = va[:].rearrange("p (j m) -> p j m", j=4)
                            nc.gpsimd.memset(va3[:, :, 64:65], 1.0)
                            kts[par] = kt
                            vas[par] = (va3, vt)
                            pw[par] = t1
                            # inline v-transposes: keeps t1's "ps"-rotation
                            # steal compact (one short window per parity)
                            rows = (slice(64, 128) if par == 0
                                    else slice(0, 64))
                            for j in range(4):
                                pvt = t1[:, 512 + 65 * j:512 + 65 * j + 65]
                                nc.tensor.matmul(
                                    pvt[:, 0:64],
                                    vt[rows, j * 128:(j + 1) * 128],
                                    ident[rows, rows],
                                    start=True, stop=True,
                                    skip_group_check=True)
                                nc.vector.tensor_copy(va3[:, j, 0:64],
                                                      pvt[:, 0:64])
                        return f

                    for par in range(2):
                        thunks.append(load(par))
                    return thunks, kts, vas

                def emit_pv(prev):
                    """PV matmuls for a previously-exp'd pair (one-pair SW
                    pipeline keeps the PE from stalling on the current exp)."""
                    u, pvas, pcp, ptch, ppb = prev
                    if pvlast and pcp != NCP - 1:
                        return
                    first = ((NCP - 1 if pvlast else 0) == pcp and ppb == 0)
                    last = (pcp == NCP - 1 and ppb == 3)
                    nc.tensor.matmul(po[ptch][0:65, :],
                                     pvas[0][:, ppb, :], u[:, 0:512],
                                     start=first, stop=False,
                                     skip_group_check=True)
                    nc.tensor.matmul(po[ptch][0:65, :],
                                     pvas[1][:, ppb, :], u[:, 512:1024],
                                     start=False, stop=last,
                                     skip_group_check=True)

                if cur is None:
                    cur = make_kv_thunks(0)
                    for th in cur[0]:
                        th()

                if wide:
                    # granule = one [128s x 512t] scores block, keyed
                    # (cp, tch, pb, par); 3 granules share one exp tile.
                    acc = [accp.tile([128, 512], f32, tag=f"acc{t}",
                                     name=f"acc{t}") for t in range(NTCH)]
                    open_ptv = {}

                    def tail_tch(tch):
                        """Normalize + store one t-chunk; emitted as soon as
                        its final PV spill lands so it overlaps the rest of
                        the exp stream instead of trailing the rep."""
                        ob = obp.tile([128, 4 * 64], f32, tag="ob")
                        ob3 = ob[:].rearrange("p (j d) -> p j d", j=4)
                        for j in range(4):
                            pt = psp.tile([128, 65], f32, tag="ps",
                                          name="pt")
                            nc.tensor.matmul(
                                pt[:],
                                acc[tch][0:65, j * 128:(j + 1) * 128],
                                identf[0:65, 0:65],
                                start=True, stop=True)
                            rr = rp.tile([128, 1], f32, tag="r")
                            nc.vector.reciprocal(rr[:], pt[:, 64:65])
                            nc.vector.tensor_scalar_mul(ob3[:, j, :],
                                                        pt[:, 0:64], rr[:])
                        nc.sync.dma_start(out=out_v[tch], in_=ob3)

                    def emit_pv_tile(entry):
                        u, gr_list = entry
                        for j, (gcp, gtch, gpb, gpar, gva) in \
                                enumerate(gr_list):
                            first = (gpb == 0 and gpar == 0)
                            if first:
                                open_ptv[(gcp, gtch)] = ptvp.tile(
                                    [128, 512], f32, tag="ptv",
                                    name="ptv")
                            ptv = open_ptv[(gcp, gtch)]
                            stop = (gpb == 3 and gpar == 1)
                            nc.tensor.matmul(
                                ptv[0:65, :], gva[:, gpb, :],
                                u[:, 512 * j:512 * (j + 1)],
                                start=first, stop=stop,
                                skip_group_check=True)
                            if stop:
                                del open_ptv[(gcp, gtch)]
                                if gcp == 0:
                                    nc.vector.tensor_copy(
                                        acc[gtch][0:65, :], ptv[0:65, :])
                                else:
                                    nc.vector.tensor_add(
                                        acc[gtch][0:65, :],
                                        acc[gtch][0:65, :], ptv[0:65, :])

                    GW = 3
                    n_gr = NCP * NTCH * 8
                    pend = []
                    tile_ps = None
                    gr_list = []
                    kts = vas = None
                    nxt = None
                    n_thunks = ti = 0
                    for gi in range(n_gr):
                        cp, w = divmod(gi, 32)
                        tch, r = divmod(w, 8)
                        pb, par = divmod(r, 2)
                        if w == 0:
                            _, kts, vas = cur
                            if cp + 1 < NCP:
                                nxt = make_kv_thunks(cp + 1)
                            else:
                                nxt0 = make_kv_thunks(0)
                                nxt = (phase_q_thunks(qt2_next) + nxt0[0],
                                       nxt0[1], nxt0[2])
                            n_thunks = len(nxt[0])
                            ti = 0
                        if tile_ps is None:
                            tile_ps = psp.tile([128, GW * 512], f32,
                                               tag="ps")
                            gr_list = []
                        col = 512 * len(gr_list)
                        rows = slice(0, 64) if par == 0 else slice(64, 128)
                        nc.tensor.matmul(
                            tile_ps[:, col:col + 512],
                            kts[par][rows, pb * 128:(pb + 1) * 128],
                            qt2[rows, tch * 512:(tch + 1) * 512],
                            start=True, stop=True)
                        gr_list.append((cp, tch, pb, par, vas[par][0]))
                        if len(gr_list) == GW or gi == n_gr - 1:
                            wcols = 512 * len(gr_list)
                            u = up.tile([128, GW * 512], bf16, tag="u")
                            nc.scalar.activation(u[:, 0:wcols],
                                                 tile_ps[:, 0:wcols],
                                                 Exp, scale=SCALE)
                            pend.append((u, gr_list))
                            tile_ps = None
                            if len(pend) > (2 if deep else 1):
                                emit_pv_tile(pend.pop(0))
                        tgt = min(n_thunks,
                                  (n_thunks * (w + 1)) // (24 if deep else 12))
                        while ti < tgt:
                            nxt[0][ti]()
                            ti += 1
                        if w == 31:
                            while ti < n_thunks:
                                nxt[0][ti]()
                                ti += 1
                            cur = nxt
                    for entry in pend:
                        emit_pv_tile(entry)
                    # tail: normalize straight from the SBUF accumulators.
                    # (Emitting each tail inline right after its final PV
                    # spill was tried and measured ~5µs SLOWER: the 16 extra
                    # "ps"-pool steals land in the rep-end region that
                    # already interleaves next-rep Q and kv thunks.)
                    for tch in range(NTCH):
                        tail_tch(tch)
                    continue

                prevs = []      # depth-2 exp->PV pipeline: PV never waits exp
                for cp in range(NCP):
                    _, kts, vas = cur
                    if nokv:
                        nxt = ([], kts, vas)
                    elif cp + 1 < NCP:
                        nxt = make_kv_thunks(cp + 1)
                    else:
                        # final pair: interleave next rep's Q projection and
                        # its s-pair 0 load instead of kv thunks
                        nxt0 = make_kv_thunks(0)
                        nxt = (phase_q_thunks(qt2_next) + nxt0[0],
                               nxt0[1], nxt0[2])
                    n_thunks = len(nxt[0])
                    ti = 0
                    pair_idx = 0
                    for tch in range(NTCH):
                        for pb in range(4):
                            ps = psp.tile([128, 1024], f32, tag="ps")
                            nc.tensor.matmul(
                                ps[:, 0:512],
                                kts[0][0:64, pb * 128:(pb + 1) * 128],
                                qt2[0:64, tch * 512:(tch + 1) * 512],
                                start=True, stop=True)
                            nc.tensor.matmul(
                                ps[:, 512:1024],
                                kts[1][64:128, pb * 128:(pb + 1) * 128],
                                qt2[64:128, tch * 512:(tch + 1) * 512],
                                start=True, stop=True)
                            if noexp:
                                u = u_ones
                            else:
                                u = up.tile([128, 1024], bf16, tag="u")
                                nc.scalar.activation(u[:], ps[:], Exp,
                                                     scale=SCALE)
                            prevs.append((u, (vas[0][0], vas[1][0]),
                                          cp, tch, pb))
                            if len(prevs) > 2:
                                emit_pv(prevs.pop(0))
                            pair_idx += 1
                            # interleave next chunk-pair's kv work over the
                            # first half of this pair (bunch: defer it all to
                            # the pair boundary)
                            target = (0 if bunch
                                      else min(n_thunks,
                                               (n_thunks * pair_idx) // 8))
                            while ti < target:
                                nxt[0][ti]()
                                ti += 1
                    while ti < n_thunks:
                        nxt[0][ti]()
                        ti += 1
                    cur = nxt
                for p in prevs:
                    emit_pv(p)

                # ---- Tail: normalize + store
                for tch in range(NTCH):
                    ot = otp.tile([128, 512], f32, tag="ot")
                    nc.vector.tensor_copy(ot[0:65, :], po[tch][0:65, :])
                    ob = obp.tile([128, 4 * 64], f32, tag="ob")
                    ob3 = ob[:].rearrange("p (j d) -> p j d", j=4)
                    for j in range(4):
                        pt = psp.tile([128, 65], f32, tag="ps", name="pt")
                        nc.tensor.matmul(pt[:],
                                         ot[0:65, j * 128:(j + 1) * 128],
                                         identf[0:65, 0:65],
                                         start=True, stop=True)
                        r = rp.tile([128, 1], f32, tag="r")
                        nc.vector.reciprocal(r[:], pt[:, 64:65])
                        nc.vector.tensor_scalar_mul(ob3[:, j, :],
                                                    pt[:, 0:64], r[:])
                    nc.sync.dma_start(out=out_v[tch], in_=ob3)

    nc.compile()
    return nc


def _get_nc(reps=1, **kw):
    key = (reps, tuple(sorted(kw.items())))
    if key not in _CACHE:
        _CACHE[key] = _build(reps, **kw)
    return _CACHE[key]


def _bf16(a):
    import ml_dtypes
    return np.asarray(a).astype(ml_dtypes.bfloat16)


def _pack_act(a, nch):
    """[L, C] row-major -> [L/512, 128, KT, 512] (chunk, partition, k, col)."""
    return np.ascontiguousarray(
        a.reshape(nch, 512, KT, 128).transpose(0, 3, 2, 1))


def _pack_w(w2):
    """[C, 128] -> [128, KT, 128]."""
    return np.ascontiguousarray(w2.reshape(KT, 128, 128).transpose(1, 0, 2))


def _prep_inputs(x, encode_out, Wq, Wk, Wv):
    x = _bf16(x)
    encode_out = _bf16(encode_out)
    Wq = _bf16(Wq)
    Wk = _bf16(Wk)
    Wv = _bf16(Wv)
    wqq = _pack_w(np.concatenate([Wq, Wq], axis=1))
    wkv = _pack_w(np.concatenate([Wk, Wv], axis=1))
    wvk = _pack_w(np.concatenate([Wv, Wk], axis=1))
    encTs = [_pack_act(encode_out[b], S // 512) for b in range(B)]
    in_maps = []
    for core in range(NCORE):
        b, th = divmod(core, 2)
        xTi = _pack_act(x[b, th * TSH:(th + 1) * TSH, :], NTCH)
        in_maps.append({"xT": xTi, "encT": encTs[b],
                        "Wqq": wqq, "Wkv": wkv, "Wvk": wvk})
    return in_maps


def kernel(x, encode_out, Wq, Wk, Wv):
    from concourse.bass_utils import run_bass_kernel_spmd
    nc = _get_nc(1, **BEST_KW)
    in_maps = _prep_inputs(x, encode_out, Wq, Wk, Wv)
    res = run_bass_kernel_spmd(nc, in_maps, list(range(NCORE)))
    out = np.empty((B, T, HS), dtype=np.float32)
    for core in range(NCORE):
        b, th = divmod(core, 2)
        o = res.results[core]["out"]            # [4, 128, 4, 64]
        out[b, th * TSH:(th + 1) * TSH] = (
            o.transpose(0, 2, 1, 3).reshape(TSH, HS))
    return out

